# revision 1
# baseline (speedup 1.0000x reference)
"""Trainium2 Bass kernel for nn_DeformableBlock (deformable attention block).

Algorithm (per core = one batch element, data-parallel over batch):
  1. PE: femb[l] = feat_l^T @ embed_w[l]  (project feature maps once, 32-dim)
     stored to DRAM as paired rows femb2[r] = [femb[r], femb[r+W]] so ONE
     512B dma_gather descriptor fetches all 4 bilinear corners of a point.
  2. PE: per 128-query block, transpose x tile and compute attn/offset logits.
  3. DVE/ACT: softmax over samples, tanh offsets, positions, floor via the
     RNE magic-constant trick, per-corner weights with zero-padding edge
     logic folded in, flat int16 indices.
  4. DMA: partition-fold indices into dma_gather's wrapped [16, N/16] layout,
     then one dma_gather per block (4096 descriptors, elem 128 f32).
  5. DVE: weighted 4-corner combine + sample-sum tree + embed bias.
"""

import sys

for _p in ("/opt/trn_rl_repo",):
    if _p not in sys.path:
        sys.path.insert(0, _p)

import numpy as np
from contextlib import ExitStack

import concourse.bass as bass
import concourse.bacc as bacc
import concourse.tile as tile
from concourse import mybir
from concourse.bass import AP
from concourse.bass_utils import run_bass_kernel_spmd
from concourse.masks import make_identity

F32 = mybir.dt.float32
I16 = mybir.dt.int16
AF = mybir.ActivationFunctionType
OP = mybir.AluOpType

B, L, P, C = 8, 4, 1024, 256
NH, NS, HD = 8, 4, 32
LEVEL_HW = [(64, 64), (32, 32), (16, 16), (8, 8)]
NQ = L * P          # queries per core
QB = NQ // 128      # 32 query blocks of 128
BPL = QB // L       # 8 blocks per level
RNE_M = 12582912.0  # 1.5*2^23; f+M lands in [2^23,2^24) where ulp==1


def _ap(t, offset, dims):
    """Raw AP on a DRAM tensor: offset and strides in flat elements."""
    return AP(tensor=t.tensor if isinstance(t, AP) else t, offset=offset,
              ap=[list(d) for d in dims])


def sv(t: AP, off: int, dims):
    """Strided free-dim view of an SBUF tile: keeps the partition dim,
    offsets `off` elements into each partition's free space."""
    base = t[:] if not isinstance(t, AP) else t
    pstride, nparts = base.ap[0]
    return AP(tensor=base.tensor, offset=base.offset + off,
              ap=[[pstride, nparts]] + [list(d) for d in dims])


def emit_kernel(ctx: ExitStack, tc: tile.TileContext, io: dict):
    nc = tc.nc
    x, ref = io["x"], io["ref"]
    feats = [io[f"feat{i}"] for i in range(L)]
    w_attn, b_attn = io["w_attn"], io["b_attn"]
    w_off, b_off = io["w_off"], io["b_off"]
    embed_w, embed_b = io["embed_w"], io["embed_b"]
    out = io["out"]
    femb2 = io["femb2"]  # 4 dram scratch tensors [(HW+8), 64] f32

    keep = ctx.enter_context(tc.tile_pool(name="keep", bufs=1))

    # ---- long-lived constants ----
    ident = keep.tile([128, 128], F32)
    make_identity(nc, ident)
    wcat = keep.tile([128, 2, 96], F32)  # k-halves of [w_attn | w_off]
    for k in range(2):
        nc.sync.dma_start(out=wcat[:, k, 0:32], in_=w_attn[k * 128:(k + 1) * 128, :])
        nc.sync.dma_start(out=wcat[:, k, 32:96], in_=w_off[k * 128:(k + 1) * 128, :])
    bias96 = keep.tile([128, 96], F32)
    nc.sync.dma_start(out=bias96[:, 0:32], in_=_ap(b_attn, 0, [[0, 128], [1, 32]]))
    nc.sync.dma_start(out=bias96[:, 32:96], in_=_ap(b_off, 0, [[0, 128], [1, 64]]))
    ebt = keep.tile([128, L, HD], F32)
    nc.sync.dma_start(out=ebt[:], in_=_ap(embed_b, 0, [[0, 128], [1, L * HD]]))
    c4 = keep.tile([128, QB * 128], F32)    # corner coefs (g, pt, sub, yc)
    idxw = keep.tile([128, QB * 256], I16)  # wrapped dma_gather indices
    zsrc = keep.tile([128, 64], F32)
    nc.vector.memset(zsrc[:], 0.0)
    permP = keep.tile([128, 128], F32)
    nc.sync.dma_start(out=permP[:], in_=io["permP"][:])
    lg_all = keep.tile([128, QB, 96], F32)
    refc = keep.tile([128, QB * 2], F32)
    nc.sync.dma_start(out=refc[:], in_=_ap(ref, 0, [[2, 128], [256, QB], [1, 2]]))
    ps = ctx.enter_context(tc.tile_pool(name="ps", bufs=3, space="PSUM"))
    ps2 = ctx.enter_context(tc.tile_pool(name="ps2", bufs=2, space="PSUM"))

    # ======== per-level pipeline: femb -> logits -> prep -> fold -> gather ====
    with ExitStack() as p1:
        fpool = p1.enter_context(tc.tile_pool(name="fpool", bufs=1))
        fsm = p1.enter_context(tc.tile_pool(name="fsm", bufs=2))
        blockio = p1.enter_context(tc.tile_pool(name="blockio", bufs=4))
        prep = p1.enter_context(tc.tile_pool(name="prep", bufs=1))
        gpool = p1.enter_context(tc.tile_pool(name="gpool", bufs=4))
        opool = p1.enter_context(tc.tile_pool(name="opool", bufs=2))
        xf = x.rearrange("l p c -> (l p) c")

        for lv, (H, W) in enumerate(LEVEL_HW):
            HW = H * W
            MT = (HW + 127) // 128
            g0 = lv * BPL
            # ---- femb2 for this level ----
            nc.sync.dma_start(
                out=_ap(femb2[lv], (HW - W) * 64 + 32, [[64, W], [1, 32]]),
                in_=zsrc[0:W, 0:32],
            )
            nc.sync.dma_start(
                out=_ap(femb2[lv], HW * 64, [[1, 64]]),
                in_=zsrc[0:1, 0:64],
            )
            fsb = fpool.tile([128, 2, HW], F32, tag="feat")
            fl = feats[lv].rearrange("c h w -> c (h w)")
            for k in range(2):
                nc.sync.dma_start(out=fsb[:, k, :], in_=fl[k * 128:(k + 1) * 128, :])
            ew = fsm.tile([128, 2, HD], F32, tag="ew")
            for k in range(2):
                nc.sync.dma_start(out=ew[:, k, :],
                                  in_=embed_w[lv, k * 128:(k + 1) * 128, :])
            fe = fsm.tile([128, MT * HD], F32, tag="fe")
            for m in range(MT):
                mp = min(128, HW - m * 128)
                psf = ps2.tile([128, HD], F32, tag="psA")
                for k in range(2):
                    nc.tensor.matmul(
                        psf[:mp, :], lhsT=fsb[:, k, m * 128:m * 128 + mp],
                        rhs=ew[:, k, :], start=(k == 0), stop=(k == 1),
                    )
                nc.scalar.copy(fe[:mp, m * HD:(m + 1) * HD], psf[:mp, :])
            f2 = femb2[lv]
            if HW >= 128:
                nc.sync.dma_start(
                    out=_ap(f2, 0, [[64, 128], [8192, MT], [1, 32]]),
                    in_=sv(fe, 0, [[32, MT], [1, 32]]),
                )
                nc.sync.dma_start(
                    out=_ap(f2, 32, [[64, 128 - W], [1, 32]]),
                    in_=fe[W:128, 0:32],
                )
                if MT > 1:
                    nc.sync.dma_start(
                        out=_ap(f2, (128 - W) * 64 + 32,
                                [[64, 128], [8192, MT - 1], [1, 32]]),
                        in_=sv(fe, 32, [[32, MT - 1], [1, 32]]),
                    )
            else:  # l3: HW=64 rows, W=8
                nc.sync.dma_start(
                    out=_ap(f2, 0, [[64, HW], [1, 32]]),
                    in_=fe[0:HW, 0:32],
                )
                nc.sync.dma_start(
                    out=_ap(f2, 32, [[64, HW - W], [1, 32]]),
                    in_=fe[W:HW, 0:32],
                )

            # ---- logits for this level's blocks ----
            for g in range(g0, g0 + BPL):
                xq = blockio.tile([128, 256], F32, tag="xq")
                nc.sync.dma_start(out=xq[:], in_=xf[g * 128:(g + 1) * 128, :])
                xt = blockio.tile([128, 2, 128], F32, tag="xt")
                for k in range(2):
                    pt_ = ps.tile([128, 128], F32, tag="ptr")
                    nc.tensor.transpose(pt_[:], xq[:, k * 128:(k + 1) * 128],
                                        ident[:])
                    nc.scalar.copy(xt[:, k, :], pt_[:])
                lg = ps2.tile([128, 96], F32, tag="plg")
                for k in range(2):
                    nc.tensor.matmul(lg[:], lhsT=xt[:, k, :], rhs=wcat[:, k, :],
                                     start=(k == 0), stop=(k == 1))
                nc.scalar.copy(lg_all[:, g, :], lg[:])
            nc.vector.tensor_add(
                lg_all[:, g0:g0 + BPL, :], lg_all[:, g0:g0 + BPL, :],
                sv(bias96, 0, [[0, BPL], [1, 96]]))

            # ---- prep for this level ----
            kap = 0.5 * (W - 1)
            ea = prep.tile([128, 256], F32, tag="ea")
            nc.scalar.activation(
                ea[:], sv(lg_all, g0 * 96, [[96, BPL], [1, 32]]), AF.Exp)
            s2 = prep.tile([128, 128], F32, tag="s2")
            nc.vector.tensor_add(s2[:], sv(ea, 0, [[4, 64], [1, 2]]),
                                 sv(ea, 2, [[4, 64], [1, 2]]))
            s1 = prep.tile([128, 64], F32, tag="s1")
            nc.vector.tensor_add(s1[:], sv(s2, 0, [[2, 64]]),
                                 sv(s2, 1, [[2, 64]]))
            dinv = prep.tile([128, 64], F32, tag="dinv")
            nc.vector.reciprocal(dinv[:], s1[:])
            a_h = prep.tile([128, 256], F32, tag="a_h")
            nc.vector.tensor_mul(a_h[:], ea[:],
                                 sv(dinv, 0, [[1, 64], [0, 4]]))

            T1 = prep.tile([128, 512], F32, tag="T1")
            nc.scalar.activation(
                T1[:], sv(lg_all, g0 * 96 + 32, [[96, BPL], [1, 64]]), AF.Tanh)
            nc.vector.tensor_add(T1[:], T1[:],
                                 sv(refc, g0 * 2, [[2, BPL], [0, 32], [1, 2]]))
            nc.scalar.activation(T1[:], T1[:], AF.Copy, bias=kap, scale=kap)
            T2 = prep.tile([128, 512], F32, tag="T2")
            nc.scalar.activation(T2[:], T1[:], AF.Copy, bias=RNE_M)
            nc.scalar.activation(T2[:], T2[:], AF.Copy, bias=-RNE_M)
            T3 = prep.tile([128, 512], F32, tag="T3")
            nc.vector.tensor_tensor(T3[:], T2[:], T1[:], OP.is_gt)
            nc.vector.tensor_tensor(T2[:], T2[:], T3[:], OP.subtract)   # x0f
            nc.vector.tensor_tensor(T3[:], T1[:], T2[:], OP.subtract)   # w1f
            nc.scalar.activation(T1[:], T3[:], AF.Copy, bias=1.0, scale=-1.0)
            T4 = prep.tile([128, 512], F32, tag="T4")  # xb
            nc.vector.tensor_scalar(T4[:], T2[:], 0.0, float(W - 2),
                                    OP.max, OP.min)
            nc.vector.tensor_tensor(T2[:], T2[:], T4[:], OP.subtract)   # d
            T5 = prep.tile([128, 512], F32, tag="T5")  # e0 -> wB
            nc.vector.tensor_scalar(T5[:], T2[:], 0.0, None, OP.is_equal)
            T6 = prep.tile([128, 512], F32, tag="T6")  # em1
            nc.vector.tensor_scalar(T6[:], T2[:], -1.0, None, OP.is_equal)
            nc.vector.tensor_scalar(T2[:], T2[:], 1.0, None, OP.is_equal)
            T7 = prep.tile([128, 512], F32, tag="T7")  # wA
            nc.vector.tensor_tensor(T7[:], T1[:], T5[:], OP.mult)
            nc.vector.tensor_tensor(T6[:], T3[:], T6[:], OP.mult)
            nc.vector.tensor_add(T7[:], T7[:], T6[:])
            nc.vector.tensor_tensor(T5[:], T3[:], T5[:], OP.mult)
            nc.vector.tensor_tensor(T2[:], T1[:], T2[:], OP.mult)
            nc.vector.tensor_add(T5[:], T5[:], T2[:])

            fly = prep.tile([128, 256], F32, tag="fly")
            nc.vector.tensor_scalar_mul(fly[:], sv(T4, 1, [[2, 256]]), float(W))
            nc.vector.tensor_add(fly[:], fly[:], sv(T4, 0, [[2, 256]]))
            T2i = prep.tile([128, 2, 128], I16, tag="T2i")
            for j in range(2):
                pf = ps.tile([128, 128], F32, tag="ptr")
                nc.tensor.matmul(pf[:], lhsT=fly[:, j * 128:(j + 1) * 128],
                                 rhs=permP[:], start=True, stop=True)
                nc.vector.tensor_copy(T2i[:, j, :], pf[:])
            for j in range(2):
                for ql in range(16):
                    nc.sync.dma_start(
                        out=sv(idxw[ql:ql + 1, :], lv * 2048 + j * 1024,
                               [[8, 128], [1, 8]]),
                        in_=T2i[:, j, ql * 8:(ql + 1) * 8],
                    )
            for t in range(1, 8):
                nc.sync.dma_start(
                    out=idxw[t * 16:(t + 1) * 16, lv * 2048:(lv + 1) * 2048],
                    in_=idxw[0:16, lv * 2048:(lv + 1) * 2048])

            wxa = prep.tile([128, 256], F32, tag="wxa")
            nc.vector.tensor_mul(wxa[:], sv(T7, 0, [[2, 256]]), a_h[:])
            wxb = prep.tile([128, 256], F32, tag="wxb")
            nc.vector.tensor_mul(wxb[:], sv(T5, 0, [[2, 256]]), a_h[:])
            for si, wx in ((0, wxa), (1, wxb)):
                for yi, wy in ((0, T7), (1, T5)):
                    nc.vector.tensor_mul(
                        sv(c4, lv * 1024 + si * 2 + yi, [[4, 256]]),
                        wx[:],
                        sv(wy, 1, [[2, 256]]),
                    )

            # ---- gather + combine for this level's blocks ----
            for g in range(g0, g0 + BPL):
                gb = gpool.tile([128, 32, 128], F32, tag="gb")
                for c in range(4):
                    nc.gpsimd.dma_gather(
                        gb[:, c * 8:(c + 1) * 8, :],
                        _ap(femb2[lv], 0, [[64, HW], [1, 128]]),
                        idxw[:, g * 256 + c * 64: g * 256 + (c + 1) * 64],
                        1024,
                        1024,
                        128,
                        elem_step=64,
                        queue_num=c,
                    )
                nc.vector.tensor_mul(
                    gb[:], gb[:],
                    sv(c4, g * 128, [[1, 128], [0, 32]]),
                )
                # reduction tree reuses gb regions (reads lead writes)
                nc.vector.tensor_add(
                    sv(gb, 0, [[1, 2048]]),
                    sv(gb, 0, [[64, 64], [1, 32]]),
                    sv(gb, 32, [[64, 64], [1, 32]]),
                )
                nc.vector.tensor_add(
                    sv(gb, 2048, [[1, 1024]]),
                    sv(gb, 0, [[64, 32], [1, 32]]),
                    sv(gb, 32, [[64, 32], [1, 32]]),
                )
                nc.vector.tensor_add(
                    sv(gb, 3072, [[1, 512]]),
                    sv(gb, 2048, [[128, 8], [1, 64]]),
                    sv(gb, 2048 + 64, [[128, 8], [1, 64]]),
                )
                ob = opool.tile([128, 256], F32, tag="ob")
                nc.vector.tensor_add(
                    ob[:],
                    sv(gb, 3072, [[64, 8], [1, 32]]),
                    sv(gb, 3072 + 32, [[64, 8], [1, 32]]),
                )
                nc.vector.tensor_add(ob[:], ob[:],
                                     sv(ebt, lv * HD, [[0, 8], [1, 32]]))
                nc.sync.dma_start(
                    out=_ap(out, g * 128 * 256, [[256, 128], [1, 256]]),
                    in_=ob[:],
                )


def build_program():
    nc = bacc.Bacc("TRN2", target_bir_lowering=False, debug=False,
                   num_swdge_queues=4)
    io = {}
    io["x"] = nc.dram_tensor("x", [L, P, C], F32, kind="ExternalInput").ap()
    io["ref"] = nc.dram_tensor("ref", [L, P, 2], F32, kind="ExternalInput").ap()
    for i, (H, W) in enumerate(LEVEL_HW):
        io[f"feat{i}"] = nc.dram_tensor(f"feat{i}", [C, H, W], F32,
                                        kind="ExternalInput").ap()
    io["w_attn"] = nc.dram_tensor("w_attn", [C, NH * NS], F32,
                                  kind="ExternalInput").ap()
    io["b_attn"] = nc.dram_tensor("b_attn", [NH * NS], F32,
                                  kind="ExternalInput").ap()
    io["w_off"] = nc.dram_tensor("w_off", [C, 2 * NH * NS], F32,
                                 kind="ExternalInput").ap()
    io["b_off"] = nc.dram_tensor("b_off", [2 * NH * NS], F32,
                                 kind="ExternalInput").ap()
    io["embed_w"] = nc.dram_tensor("embed_w", [L, C, HD], F32,
                                   kind="ExternalInput").ap()
    io["embed_b"] = nc.dram_tensor("embed_b", [L, HD], F32,
                                   kind="ExternalInput").ap()
    io["permP"] = nc.dram_tensor("permP", [128, 128], F32,
                                 kind="ExternalInput").ap()
    io["out"] = nc.dram_tensor("out", [L, P, NH * HD], F32,
                               kind="ExternalOutput").ap()
    io["femb2"] = [
        nc.dram_tensor(f"femb2_{i}", [H * W + 8, 64], F32, kind="Internal").ap()
        for i, (H, W) in enumerate(LEVEL_HW)
    ]
    with tile.TileContext(nc) as tc:
        with ExitStack() as ctx:
            emit_kernel(ctx, tc, io)
    nc.compile()
    return nc


_prog = None


def kernel(**inputs):
    global _prog
    if _prog is None:
        _prog = build_program()
    nc = _prog
    res = run_bass_kernel_spmd(nc, _in_maps(inputs), list(range(B)))
    out = np.stack([res.results[i]["out"] for i in range(B)], axis=0)
    return out.reshape(B, L, P, NH * HD)


def _perm_matrix():
    p = np.zeros((128, 128), np.float32)
    for n in range(128):
        p[(n % 8) * 16 + n // 8, n] = 1.0
    return p


def _in_maps(inputs):
    keys = ["x", "ref", "feat0", "feat1", "feat2", "feat3",
            "w_attn", "b_attn", "w_off", "b_off", "embed_w", "embed_b"]
    per_batch = {"x", "ref", "feat0", "feat1", "feat2", "feat3"}
    pm = _perm_matrix()
    maps = []
    for b in range(B):
        m = {"permP": pm}
        for kk in keys:
            v = np.ascontiguousarray(np.asarray(inputs[kk], dtype=np.float32))
            m[kk] = v[b] if kk in per_batch else v
        maps.append(m)
    return maps


def profile(inputs):
    """Run with tracing; returns HW exec time in ns (or None if unavailable)."""
    global _prog
    if _prog is None:
        _prog = build_program()
    res = run_bass_kernel_spmd(_prog, _in_maps(inputs), list(range(B)), trace=True)
    return res.exec_time_ns


if __name__ == "__main__":
    build_program()
    print("build ok")



# revision 26
# speedup vs baseline: 1.4659x; 1.4659x over previous
"""Trainium2 Bass kernel for nn_DeformableBlock (deformable attention block).

Per core = one batch element (data-parallel over batch). Two gather paths:

Levels 0,1 (64x64 / 32x32): descriptor gather.
  femb4[r] = [femb[r], femb[r+W], femb[r+1], femb[r+W+1]] packed fp16 rows of
  256B so ONE dma_gather descriptor fetches all 4 bilinear corners of a point.
  One 4096-descriptor dma_gather per 128-query block; fp16 combine via
  tensor_mul + two pool-avg reductions (x16 folded into the corner weights).

Levels 2,3 (16x16 / 8x8): no gather at all.
  Bilinear+attention weights are evaluated DENSELY over the small grid as a
  separable hat product A[q, (Y,X)] = sum_s attn_s*hat(Y-fy_s)*hat(X-fx_s)
  (exactly equal to zero-padding bilinear), then out = A @ femb via PE
  matmuls (A transposed on PE). Removes half the SWDGE descriptor-generation
  serial cost, which dominates the descriptor-gather path.

fp16 is used for feature values/gather/combine (8x finer rounding than bf16
at the same 2 bytes); positions/weights math stays fp32.
"""

import os
import sys

for _p in ("/opt/trn_rl_repo",):
    if _p not in sys.path:
        sys.path.insert(0, _p)

SKIP_DENSE = bool(os.environ.get("SKIP_DENSE"))
SKIP_GATHER = bool(os.environ.get("SKIP_GATHER"))

import numpy as np
from contextlib import ExitStack

import concourse.bass as bass
import concourse.bacc as bacc
import concourse.tile as tile
from concourse import mybir
from concourse.bass import AP
from concourse.bass_utils import run_bass_kernel_spmd
from concourse.masks import make_identity

F32 = mybir.dt.float32
F16 = mybir.dt.float16
I16 = mybir.dt.int16
AF = mybir.ActivationFunctionType
OP = mybir.AluOpType
PF = mybir.PoolFunctionType

B, L, P, C = 8, 4, 1024, 256
NH, NS, HD = 8, 4, 32
LEVEL_HW = [(64, 64), (32, 32), (16, 16), (8, 8)]
NQ = L * P          # queries per core
QB = NQ // 128      # 32 query blocks of 128
BPL = QB // L       # 8 blocks per level
RNE_M = 12582912.0  # 1.5*2^23; f+M lands in [2^23,2^24) where ulp==1
DESC_LV = (0, 1)    # descriptor-gather levels
DENSE_LV = (2, 3)   # PE dense-hat levels


def _ap(t, offset, dims):
    """Raw AP on a DRAM tensor: offset and strides in flat elements."""
    return AP(tensor=t.tensor if isinstance(t, AP) else t, offset=offset,
              ap=[list(d) for d in dims])


def sv(t: AP, off: int, dims):
    """Strided free-dim view of an SBUF tile: keeps the partition dim,
    offsets `off` elements into each partition's free space."""
    base = t[:] if not isinstance(t, AP) else t
    pstride, nparts = base.ap[0]
    return AP(tensor=base.tensor, offset=base.offset + off,
              ap=[[pstride, nparts]] + [list(d) for d in dims])


def emit_kernel(ctx: ExitStack, tc: tile.TileContext, io: dict):
    nc = tc.nc
    x, ref = io["x"], io["ref"]
    feats = [io[f"feat{i}"] for i in range(L)]
    w_attn, b_attn = io["w_attn"], io["b_attn"]
    w_off, b_off = io["w_off"], io["b_off"]
    embed_w, embed_b = io["embed_w"], io["embed_b"]
    out = io["out"]
    femb4 = io["femb4"]  # dram scratch [HW, 128] f16 for levels 0,1

    keep = ctx.enter_context(tc.tile_pool(name="keep", bufs=1))

    # ---- long-lived constants ----
    identH = keep.tile([128, 128], F16)
    make_identity(nc, identH)
    wcat = keep.tile([128, 2, 96], F32)  # k-halves of [w_attn | w_off]
    for k in range(2):
        nc.sync.dma_start(out=wcat[:, k, 0:32], in_=w_attn[k * 128:(k + 1) * 128, :])
        nc.sync.dma_start(out=wcat[:, k, 32:96], in_=w_off[k * 128:(k + 1) * 128, :])
    wcat16 = keep.tile([128, 2, 96], F16)
    nc.scalar.copy(sv(wcat16, 0, [[1, 192]]), sv(wcat, 0, [[1, 192]]))
    bias96 = keep.tile([128, 96], F32)
    nc.sync.dma_start(out=bias96[:, 0:32], in_=_ap(b_attn, 0, [[0, 128], [1, 32]]))
    nc.sync.dma_start(out=bias96[:, 32:96], in_=_ap(b_off, 0, [[0, 128], [1, 64]]))
    ebt = keep.tile([128, L, HD], F32)
    nc.sync.dma_start(out=ebt[:], in_=_ap(embed_b, 0, [[0, 128], [1, L * HD]]))
    ebt16 = keep.tile([128, L, HD], F16)
    nc.scalar.copy(sv(ebt16, 0, [[1, L * HD]]), sv(ebt, 0, [[1, L * HD]]))
    c4 = keep.tile([128, 2 * 1024], F32)    # corner coefs, levels 0,1
    c4h = keep.tile([128, 2 * 1024], F16)   # fp16, x16 (pool-avg compensation)
    idxw = keep.tile([128, 2 * 2048], I16)  # wrapped dma_gather indices lv0,1
    permP = keep.tile([128, 128], F32)
    nc.sync.dma_start(out=permP[:], in_=io["permP"][:])
    lg_all = keep.tile([128, QB, 96], F32)
    refc = keep.tile([128, QB * 2], F32)
    nc.sync.dma_start(out=refc[:], in_=_ap(ref, 0, [[2, 128], [256, QB], [1, 2]]))
    xgt = keep.tile([128, 16], F32)
    nc.sync.dma_start(out=xgt[:], in_=_ap(io["xg"], 0, [[0, 128], [1, 16]]))
    zh = keep.tile([128, 128], F16)
    nc.vector.memset(zh[:], 0.0)
    febf2 = keep.tile([128, 2, HD], F16)
    febf3 = keep.tile([128, HD], F16)
    febf = {2: febf2, 3: febf3}  # fp16 femb tables for dense levels

    # ================= phase B: femb for all levels =================
    with ExitStack() as pb:
        fpool = pb.enter_context(tc.tile_pool(name="fpool", bufs=1))
        fsm = pb.enter_context(tc.tile_pool(name="fsm", bufs=2))
        psB = pb.enter_context(tc.tile_pool(name="psB", bufs=2, space="PSUM"))
        for lv, (H, W) in enumerate(LEVEL_HW):
            HW = H * W
            MT = (HW + 127) // 128
            fsb = fpool.tile([128, 2, HW], F32, tag="feat")
            fl = feats[lv].rearrange("c h w -> c (h w)")
            for k in range(2):
                nc.sync.dma_start(out=fsb[:, k, :], in_=fl[k * 128:(k + 1) * 128, :])
            fsb16 = fpool.tile([128, 2, HW], F16, tag="feat16")
            for k in range(2):
                nc.scalar.copy(fsb16[:, k, :], fsb[:, k, :])
            ew = fsm.tile([128, 2, HD], F32, tag="ew")
            for k in range(2):
                nc.sync.dma_start(out=ew[:, k, :],
                                  in_=embed_w[lv, k * 128:(k + 1) * 128, :])
            ew16 = fsm.tile([128, 2, HD], F16, tag="ew16")
            nc.scalar.copy(sv(ew16, 0, [[1, 2 * HD]]), sv(ew, 0, [[1, 2 * HD]]))
            if lv in DENSE_LV:
                fe16 = febf[lv]
            else:
                fe16 = fsm.tile([128, MT * HD], F16, tag="fe16")
            for m in range(MT):
                mp = min(128, HW - m * 128)
                psf = psB.tile([128, HD], F32, tag="psA")
                for k in range(2):
                    nc.tensor.matmul(
                        psf[:mp, :], lhsT=fsb16[:, k, m * 128:m * 128 + mp],
                        rhs=ew16[:, k, :], start=(k == 0), stop=(k == 1),
                    )
                if lv == 2:
                    nc.scalar.copy(fe16[:mp, m, :], psf[:mp, :])
                elif lv == 3:
                    nc.scalar.copy(fe16[:mp, :], psf[:mp, :])
                else:
                    nc.scalar.copy(fe16[:mp, m * HD:(m + 1) * HD], psf[:mp, :])
            if lv in DESC_LV:
                # femb4[r, k*32:(k+1)*32] = femb[r + sig], sig in (0, W, 1, W+1)
                f4 = femb4[lv]
                # zero the tail slivers the shifted stores leave uncovered
                # (never fetched: indices clamp to <= HW-W-2; disjoint from
                # the data stores so DMA completion order doesn't matter)
                nc.sync.dma_start(
                    out=_ap(f4, (HW - W) * 128 + 32, [[128, W], [1, 32]]),
                    in_=zh[0:W, 0:32])
                nc.sync.dma_start(
                    out=_ap(f4, (HW - 1) * 128 + 64, [[128, 1], [1, 32]]),
                    in_=zh[0:1, 0:32])
                nc.sync.dma_start(
                    out=_ap(f4, (HW - W - 1) * 128 + 96, [[128, W + 1], [1, 32]]),
                    in_=zh[0:W + 1, 0:32])
                for k, sig in enumerate((0, W, 1, W + 1)):
                    nc.sync.dma_start(
                        out=_ap(f4, k * 32, [[128, 128 - sig], [1, 32]]),
                        in_=fe16[sig:128, 0:32],
                    )
                    nc.sync.dma_start(
                        out=_ap(f4, (128 - sig) * 128 + k * 32,
                                [[128, 128], [16384, MT - 1], [1, 32]]),
                        in_=sv(fe16, HD, [[HD, MT - 1], [1, HD]]),
                    )

    # ================= phase C: logits for all blocks =================
    with ExitStack() as pc:
        xpool = pc.enter_context(tc.tile_pool(name="xpool", bufs=2))
        psT = pc.enter_context(tc.tile_pool(name="psT", bufs=2, space="PSUM"))
        psLg = pc.enter_context(tc.tile_pool(name="psLg", bufs=2, space="PSUM"))
        for lv in range(L):
            xlev = xpool.tile([128, BPL, 256], F32, tag="xlev")
            nc.sync.dma_start(
                out=xlev[:],
                in_=_ap(x, lv * P * C, [[256, 128], [128 * 256, BPL], [1, 256]]),
            )
            xh = xpool.tile([128, BPL, 256], F16, tag="xh")
            nc.scalar.copy(sv(xh, 0, [[1, BPL * 256]]), sv(xlev, 0, [[1, BPL * 256]]))
            for g in range(BPL):
                pt_ = psT.tile([128, 256], F16, tag="ptr")
                for k in range(2):
                    nc.tensor.transpose(pt_[:, k * 128:(k + 1) * 128],
                                        xh[:, g, k * 128:(k + 1) * 128], identH[:])
                xt = xpool.tile([128, 2, 128], F16, tag="xt")
                nc.scalar.copy(sv(xt, 0, [[1, 256]]), sv(pt_, 0, [[1, 256]]))
                lg = psLg.tile([128, 96], F32, tag="plg")
                for k in range(2):
                    nc.tensor.matmul(lg[:], lhsT=xt[:, k, :], rhs=wcat16[:, k, :],
                                     start=(k == 0), stop=(k == 1))
                nc.scalar.copy(lg_all[:, lv * BPL + g, :], lg[:])
            nc.vector.tensor_add(
                lg_all[:, lv * BPL:(lv + 1) * BPL, :],
                lg_all[:, lv * BPL:(lv + 1) * BPL, :],
                sv(bias96, 0, [[0, BPL], [1, 96]]))

    prep = ctx.enter_context(tc.tile_pool(name="prep", bufs=1))

    def softmax_pos(lv, W, scale16):
        """Common prep: attention softmax a_h and pixel positions T1."""
        g0 = lv * BPL
        kap = 0.5 * (W - 1)
        ea = prep.tile([128, 256], F32, tag="ea")
        nc.scalar.activation(
            ea[:], sv(lg_all, g0 * 96, [[96, BPL], [1, 32]]), AF.Exp)
        s2 = prep.tile([128, 128], F32, tag="s2")
        nc.vector.tensor_add(s2[:], sv(ea, 0, [[4, 64], [1, 2]]),
                             sv(ea, 2, [[4, 64], [1, 2]]))
        s1 = prep.tile([128, 64], F32, tag="s1")
        nc.vector.tensor_add(s1[:], sv(s2, 0, [[2, 64]]),
                             sv(s2, 1, [[2, 64]]))
        dinv = prep.tile([128, 64], F32, tag="dinv")
        nc.vector.reciprocal(dinv[:], s1[:])
        a_h = prep.tile([128, 256], F32, tag="a_h")
        nc.vector.tensor_mul(a_h[:], ea[:],
                             sv(dinv, 0, [[1, 64], [0, 4]]))
        T1 = prep.tile([128, 512], F32, tag="T1")
        nc.scalar.activation(
            T1[:], sv(lg_all, g0 * 96 + 32, [[96, BPL], [1, 64]]), AF.Tanh)
        nc.vector.tensor_add(T1[:], T1[:],
                             sv(refc, g0 * 2, [[2, BPL], [0, 32], [1, 2]]))
        nc.scalar.activation(T1[:], T1[:], AF.Copy, bias=kap, scale=kap)
        return a_h, T1

    # ============ phase D: prep + indices for descriptor levels ============
    pd_stack = ctx.enter_context(ExitStack())
    psP = pd_stack.enter_context(tc.tile_pool(name="psP", bufs=2, space="PSUM"))
    for lv in DESC_LV:
        H, W = LEVEL_HW[lv]
        g0 = lv * BPL
        a_h, T1 = softmax_pos(lv, W, None)

        T2 = prep.tile([128, 512], F32, tag="T2")
        nc.scalar.activation(T2[:], T1[:], AF.Copy, bias=RNE_M)
        nc.scalar.activation(T2[:], T2[:], AF.Copy, bias=-RNE_M)
        T3 = prep.tile([128, 512], F32, tag="T3")
        nc.vector.tensor_tensor(T3[:], T2[:], T1[:], OP.is_gt)
        nc.vector.tensor_tensor(T2[:], T2[:], T3[:], OP.subtract)   # x0f
        nc.vector.tensor_tensor(T3[:], T1[:], T2[:], OP.subtract)   # w1f
        nc.scalar.activation(T1[:], T3[:], AF.Copy, bias=1.0, scale=-1.0)
        T4 = prep.tile([128, 512], F32, tag="T4")  # xb
        nc.vector.tensor_scalar(T4[:], T2[:], 0.0, float(W - 2),
                                OP.max, OP.min)
        nc.vector.tensor_tensor(T2[:], T2[:], T4[:], OP.subtract)   # d
        T5 = prep.tile([128, 512], F32, tag="T5")  # e0 -> wB
        nc.vector.tensor_scalar(T5[:], T2[:], 0.0, None, OP.is_equal)
        T6 = prep.tile([128, 512], F32, tag="T6")  # em1
        nc.vector.tensor_scalar(T6[:], T2[:], -1.0, None, OP.is_equal)
        nc.vector.tensor_scalar(T2[:], T2[:], 1.0, None, OP.is_equal)
        T7 = prep.tile([128, 512], F32, tag="T7")  # wA
        nc.vector.tensor_tensor(T7[:], T1[:], T5[:], OP.mult)
        nc.vector.tensor_tensor(T6[:], T3[:], T6[:], OP.mult)
        nc.vector.tensor_add(T7[:], T7[:], T6[:])
        nc.vector.tensor_tensor(T5[:], T3[:], T5[:], OP.mult)
        nc.vector.tensor_tensor(T2[:], T1[:], T2[:], OP.mult)
        nc.vector.tensor_add(T5[:], T5[:], T2[:])

        fly = prep.tile([128, 256], F32, tag="fly")
        nc.vector.tensor_scalar_mul(fly[:], sv(T4, 1, [[2, 256]]), float(W))
        nc.vector.tensor_add(fly[:], fly[:], sv(T4, 0, [[2, 256]]))
        T2i = prep.tile([128, 2, 128], I16, tag="T2i")
        for j in range(2):
            pf = psP.tile([128, 128], F32, tag="ptrf")
            nc.tensor.matmul(pf[:], lhsT=fly[:, j * 128:(j + 1) * 128],
                             rhs=permP[:], start=True, stop=True)
            nc.vector.tensor_copy(T2i[:, j, :], pf[:])
        for j in range(2):
            for ql in range(16):
                nc.sync.dma_start(
                    out=sv(idxw[ql:ql + 1, :], lv * 2048 + j * 1024,
                           [[8, 128], [1, 8]]),
                    in_=T2i[:, j, ql * 8:(ql + 1) * 8],
                )
        for t in range(1, 8):
            nc.sync.dma_start(
                out=idxw[t * 16:(t + 1) * 16, lv * 2048:(lv + 1) * 2048],
                in_=idxw[0:16, lv * 2048:(lv + 1) * 2048])

        wxa = prep.tile([128, 256], F32, tag="wxa")
        nc.vector.tensor_mul(wxa[:], sv(T7, 0, [[2, 256]]), a_h[:])
        wxb = prep.tile([128, 256], F32, tag="wxb")
        nc.vector.tensor_mul(wxb[:], sv(T5, 0, [[2, 256]]), a_h[:])
        for si, wx in ((0, wxa), (1, wxb)):
            for yi, wy in ((0, T7), (1, T5)):
                nc.vector.tensor_mul(
                    sv(c4, lv * 1024 + si * 2 + yi, [[4, 256]]),
                    wx[:],
                    sv(wy, 1, [[2, 256]]),
                )
        # fp16 corner weights
        nc.scalar.copy(c4h[:, lv * 1024:(lv + 1) * 1024],
                       c4[:, lv * 1024:(lv + 1) * 1024])

    # ============ phase E: dense-hat levels (2, 3) ============
    pd_stack.close()  # release phase-D PSUM banks
    dn = ctx.enter_context(tc.tile_pool(name="dn", bufs=1))
    dn2 = ctx.enter_context(tc.tile_pool(name="dn2", bufs=2))
    psD = ctx.enter_context(tc.tile_pool(name="psD", bufs=2, space="PSUM"))
    psO = ctx.enter_context(tc.tile_pool(name="psO", bufs=2, space="PSUM"))
    opool = ctx.enter_context(tc.tile_pool(name="opool", bufs=2))

    for lv in (() if SKIP_DENSE else DENSE_LV):
        H, W = LEVEL_HW[lv]
        WW = H * W
        a_h, T1 = softmax_pos(lv, W, None)
        a_h16 = prep.tile([128, 256], F16, tag="a_h16")
        nc.scalar.copy(a_h16[:], a_h[:])

        # hat weights over the grid: relu(1 - |grid - pos|), fp32 -> fp16
        hx16 = dn.tile([128, 256, 16], F16, tag="hx16")
        hy16 = dn.tile([128, 256, 16], F16, tag="hy16")
        dxt = dn.tile([128, 256, 16], F16, tag="dxt")
        for ax, ht in ((0, hx16), (1, hy16)):
            nc.vector.tensor_tensor(
                sv(dxt, 0, [[W, 256], [1, W]]),
                sv(xgt, 0, [[0, 256], [1, W]]),
                sv(T1, ax, [[2, 256], [0, W]]),
                OP.subtract)
            nc.scalar.activation(sv(dxt, 0, [[1, 256 * W]]),
                                 sv(dxt, 0, [[1, 256 * W]]), AF.Abs)
            nc.scalar.activation(sv(ht, 0, [[1, 256 * W]]),
                                 sv(dxt, 0, [[1, 256 * W]]),
                                 AF.Relu, bias=1.0, scale=-1.0)
        # fold attn (x4) into hy
        nc.vector.tensor_mul(sv(hy16, 0, [[W, 256], [1, W]]),
                             sv(hy16, 0, [[W, 256], [1, W]]),
                             sv(a_h16, 0, [[1, 256], [0, W]]))

        for qb in range(BPL):
            tmpt = dn2.tile([128, 4, 8 * WW], F16, tag=f"tmp{lv}")
            base = qb * 32 * W
            for ns in range(NS):
                nc.vector.tensor_mul(
                    sv(tmpt, ns * 8 * WW, [[WW, 8], [W, W], [1, W]]),
                    sv(hy16, base + ns * W, [[4 * W, 8], [1, W], [0, W]]),
                    sv(hx16, base + ns * W, [[4 * W, 8], [0, W], [1, W]]),
                )
            At = dn2.tile([128, 8 * WW], F16, tag=f"At{lv}")
            nc.vector.tensor_add(sv(tmpt, 0, [[1, 8 * WW]]),
                                 sv(tmpt, 0, [[1, 8 * WW]]),
                                 sv(tmpt, 8 * WW, [[1, 8 * WW]]))
            nc.vector.tensor_add(sv(tmpt, 2 * 8 * WW, [[1, 8 * WW]]),
                                 sv(tmpt, 2 * 8 * WW, [[1, 8 * WW]]),
                                 sv(tmpt, 3 * 8 * WW, [[1, 8 * WW]]))
            nc.vector.tensor_add(sv(At, 0, [[1, 8 * WW]]),
                                 sv(tmpt, 0, [[1, 8 * WW]]),
                                 sv(tmpt, 2 * 8 * WW, [[1, 8 * WW]]))
            pso = psO.tile([128, 256], F32, tag="psout")
            for h in range(NH):
                if lv == 2:
                    pst = psD.tile([128, 256], F16, tag="pst2")
                    for cch in range(2):
                        nc.tensor.transpose(
                            pst[:, cch * 128:(cch + 1) * 128],
                            At[:, h * WW + cch * 128: h * WW + (cch + 1) * 128],
                            identH[:])
                    Ah = dn2.tile([128, 256], F16, tag="Ah2")
                    nc.scalar.copy(Ah[:], pst[:])
                    for cch in range(2):
                        nc.tensor.matmul(
                            pso[:, h * 32:(h + 1) * 32],
                            lhsT=Ah[:, cch * 128:(cch + 1) * 128],
                            rhs=febf[2][:, cch, :],
                            start=(cch == 0), stop=(cch == 1))
                else:
                    pst = psD.tile([64, 128], F16, tag="pst3")
                    nc.tensor.transpose(pst[:], At[:, h * WW:(h + 1) * WW],
                                        identH[:])
                    Ah = dn2.tile([64, 128], F16, tag="Ah3")
                    nc.scalar.copy(Ah[:], pst[:])
                    nc.tensor.matmul(
                        pso[:, h * 32:(h + 1) * 32],
                        lhsT=Ah[:], rhs=febf[3][0:64, :],
                        start=True, stop=True)
            ob = opool.tile([128, 256], F32, tag="ob")
            nc.vector.tensor_add(ob[:], pso[:],
                                 sv(ebt, lv * HD, [[0, 8], [1, 32]]))
            nc.sync.dma_start(
                out=_ap(out, (lv * BPL + qb) * 128 * 256, [[256, 128], [1, 256]]),
                in_=ob[:],
            )

    # ============ phase F: gathers + combines for levels 0,1 ============
    gpool = ctx.enter_context(tc.tile_pool(name="gpool", bufs=6))
    cpool = ctx.enter_context(tc.tile_pool(name="cpool", bufs=2))
    for lv in (() if SKIP_GATHER else DESC_LV):
        H, W = LEVEL_HW[lv]
        HW = H * W
        for g in range(BPL):
            blk = lv * BPL + g
            gb = gpool.tile([128, 32, 128], F16, tag="gb")
            for hf in range(4):
                nc.gpsimd.dma_gather(
                    gb[:, hf * 8:(hf + 1) * 8, :],
                    _ap(femb4[lv], 0, [[128, HW], [1, 128]]),
                    idxw[:, blk * 256 + hf * 64: blk * 256 + (hf + 1) * 64],
                    1024,
                    1024,
                    128,
                    elem_step=128,
                    queue_num=hf,
                )
            nc.vector.tensor_mul(
                sv(gb, 0, [[128, 32], [32, 4], [1, 32]]),
                sv(gb, 0, [[128, 32], [32, 4], [1, 32]]),
                sv(c4h, lv * 1024 + g * 128, [[4, 32], [1, 4], [0, 32]]),
            )
            # reduction tree over (corner, sample): 4096 -> 256
            ta = cpool.tile([128, 2048], F16, tag="ta")
            nc.vector.tensor_add(
                sv(ta, 0, [[64, 32], [1, 64]]),
                sv(gb, 0, [[128, 32], [1, 64]]),
                sv(gb, 64, [[128, 32], [1, 64]]))
            tb = cpool.tile([128, 1024], F16, tag="tb")
            nc.vector.tensor_add(
                sv(tb, 0, [[32, 32], [1, 32]]),
                sv(ta, 0, [[64, 32], [1, 32]]),
                sv(ta, 32, [[64, 32], [1, 32]]))
            tc2 = cpool.tile([128, 512], F16, tag="tc2")
            nc.vector.tensor_add(
                sv(tc2, 0, [[64, 8], [1, 64]]),
                sv(tb, 0, [[128, 8], [1, 64]]),
                sv(tb, 64, [[128, 8], [1, 64]]))
            ob16 = cpool.tile([128, 256], F16, tag="ob16")
            nc.vector.tensor_add(
                sv(ob16, 0, [[32, 8], [1, 32]]),
                sv(tc2, 0, [[64, 8], [1, 32]]),
                sv(tc2, 32, [[64, 8], [1, 32]]))
            nc.vector.tensor_add(
                sv(ob16, 0, [[32, 8], [1, 32]]),
                sv(ob16, 0, [[32, 8], [1, 32]]),
                sv(ebt16, lv * HD, [[0, 8], [1, 32]]))
            ob = opool.tile([128, 256], F32, tag="ob")
            nc.scalar.copy(ob[:], ob16[:])
            nc.sync.dma_start(
                out=_ap(out, blk * 128 * 256, [[256, 128], [1, 256]]),
                in_=ob[:],
            )


def build_program():
    nc = bacc.Bacc("TRN2", target_bir_lowering=False, debug=False,
                   num_swdge_queues=4)
    io = {}
    io["x"] = nc.dram_tensor("x", [L, P, C], F32, kind="ExternalInput").ap()
    io["ref"] = nc.dram_tensor("ref", [L, P, 2], F32, kind="ExternalInput").ap()
    for i, (H, W) in enumerate(LEVEL_HW):
        io[f"feat{i}"] = nc.dram_tensor(f"feat{i}", [C, H, W], F32,
                                        kind="ExternalInput").ap()
    io["w_attn"] = nc.dram_tensor("w_attn", [C, NH * NS], F32,
                                  kind="ExternalInput").ap()
    io["b_attn"] = nc.dram_tensor("b_attn", [NH * NS], F32,
                                  kind="ExternalInput").ap()
    io["w_off"] = nc.dram_tensor("w_off", [C, 2 * NH * NS], F32,
                                 kind="ExternalInput").ap()
    io["b_off"] = nc.dram_tensor("b_off", [2 * NH * NS], F32,
                                 kind="ExternalInput").ap()
    io["embed_w"] = nc.dram_tensor("embed_w", [L, C, HD], F32,
                                   kind="ExternalInput").ap()
    io["embed_b"] = nc.dram_tensor("embed_b", [L, HD], F32,
                                   kind="ExternalInput").ap()
    io["permP"] = nc.dram_tensor("permP", [128, 128], F32,
                                 kind="ExternalInput").ap()
    io["xg"] = nc.dram_tensor("xg", [16], F32, kind="ExternalInput").ap()
    io["out"] = nc.dram_tensor("out", [L, P, NH * HD], F32,
                               kind="ExternalOutput").ap()
    io["femb4"] = {
        lv: nc.dram_tensor(f"femb4_{lv}", [LEVEL_HW[lv][0] * LEVEL_HW[lv][1], 128],
                           F16, kind="Internal").ap()
        for lv in DESC_LV
    }
    with tile.TileContext(nc) as tc:
        with ExitStack() as ctx:
            emit_kernel(ctx, tc, io)
    nc.compile()
    return nc


_prog = None


def kernel(**inputs):
    global _prog
    if _prog is None:
        _prog = build_program()
    nc = _prog
    res = run_bass_kernel_spmd(nc, _in_maps(inputs), list(range(B)))
    out = np.stack([res.results[i]["out"] for i in range(B)], axis=0)
    return out.reshape(B, L, P, NH * HD)


def _perm_matrix():
    p = np.zeros((128, 128), np.float32)
    for n in range(128):
        p[(n % 8) * 16 + n // 8, n] = 1.0
    return p


def _in_maps(inputs):
    keys = ["x", "ref", "feat0", "feat1", "feat2", "feat3",
            "w_attn", "b_attn", "w_off", "b_off", "embed_w", "embed_b"]
    per_batch = {"x", "ref", "feat0", "feat1", "feat2", "feat3"}
    pm = _perm_matrix()
    xg = np.arange(16, dtype=np.float32)
    maps = []
    for b in range(B):
        m = {"permP": pm, "xg": xg}
        for kk in keys:
            v = np.ascontiguousarray(np.asarray(inputs[kk], dtype=np.float32))
            m[kk] = v[b] if kk in per_batch else v
        maps.append(m)
    return maps


def profile(inputs):
    """Run with tracing; returns HW exec time in ns (or None if unavailable)."""
    global _prog
    if _prog is None:
        _prog = build_program()
    res = run_bass_kernel_spmd(_prog, _in_maps(inputs), list(range(B)), trace=True)
    return res.exec_time_ns


if __name__ == "__main__":
    build_program()
    print("build ok")


# revision 36
# speedup vs baseline: 1.5085x; 1.0291x over previous
"""Trainium2 Bass kernel for nn_DeformableBlock (deformable attention block).

Per core = one batch element (data-parallel over batch). Two gather paths:

Levels 0,1 (64x64 / 32x32): descriptor gather.
  femb4[r] = [femb[r], femb[r+W], femb[r+1], femb[r+W+1]] packed fp16 rows of
  256B so ONE dma_gather descriptor fetches all 4 bilinear corners of a point.
  One 4096-descriptor dma_gather per 128-query block; fp16 combine via
  tensor_mul + two pool-avg reductions (x16 folded into the corner weights).

Levels 2,3 (16x16 / 8x8): no gather at all.
  Bilinear+attention weights are evaluated DENSELY over the small grid as a
  separable hat product A[q, (Y,X)] = sum_s attn_s*hat(Y-fy_s)*hat(X-fx_s)
  (exactly equal to zero-padding bilinear), then out = A @ femb via PE
  matmuls (A transposed on PE). Removes half the SWDGE descriptor-generation
  serial cost, which dominates the descriptor-gather path.

fp16 is used for feature values/gather/combine (8x finer rounding than bf16
at the same 2 bytes); positions/weights math stays fp32.
"""

import os
import sys

for _p in ("/opt/trn_rl_repo",):
    if _p not in sys.path:
        sys.path.insert(0, _p)

SKIP_DENSE = bool(os.environ.get("SKIP_DENSE"))
SKIP_GATHER = bool(os.environ.get("SKIP_GATHER"))

import numpy as np
from contextlib import ExitStack

import concourse.bass as bass
import concourse.bacc as bacc
import concourse.tile as tile
from concourse import mybir
from concourse.bass import AP
from concourse.bass_utils import run_bass_kernel_spmd
from concourse.masks import make_identity

F32 = mybir.dt.float32
F16 = mybir.dt.float16
I16 = mybir.dt.int16
AF = mybir.ActivationFunctionType
OP = mybir.AluOpType
PF = mybir.PoolFunctionType

B, L, P, C = 8, 4, 1024, 256
NH, NS, HD = 8, 4, 32
LEVEL_HW = [(64, 64), (32, 32), (16, 16), (8, 8)]
NQ = L * P          # queries per core
QB = NQ // 128      # 32 query blocks of 128
BPL = QB // L       # 8 blocks per level
RNE_M = 12582912.0  # 1.5*2^23; f+M lands in [2^23,2^24) where ulp==1
DESC_LV = (0, 1)    # descriptor-gather levels
DENSE_LV = (2, 3)   # PE dense-hat levels


def _ap(t, offset, dims):
    """Raw AP on a DRAM tensor: offset and strides in flat elements."""
    return AP(tensor=t.tensor if isinstance(t, AP) else t, offset=offset,
              ap=[list(d) for d in dims])


def sv(t: AP, off: int, dims):
    """Strided free-dim view of an SBUF tile: keeps the partition dim,
    offsets `off` elements into each partition's free space."""
    base = t[:] if not isinstance(t, AP) else t
    pstride, nparts = base.ap[0]
    return AP(tensor=base.tensor, offset=base.offset + off,
              ap=[[pstride, nparts]] + [list(d) for d in dims])


def emit_kernel(ctx: ExitStack, tc: tile.TileContext, io: dict):
    nc = tc.nc
    x, ref = io["x"], io["ref"]
    feats = [io[f"feat{i}"] for i in range(L)]
    w_attn, b_attn = io["w_attn"], io["b_attn"]
    w_off, b_off = io["w_off"], io["b_off"]
    embed_w, embed_b = io["embed_w"], io["embed_b"]
    out = io["out"]
    femb4 = io["femb4"]  # dram scratch [HW, 128] f16 for levels 0,1

    keep = ctx.enter_context(tc.tile_pool(name="keep", bufs=1))

    # ---- long-lived constants ----
    identH = keep.tile([128, 128], F16)
    make_identity(nc, identH)
    wcat = keep.tile([128, 2, 96], F32)  # k-halves of [w_attn | w_off]
    for k in range(2):
        nc.sync.dma_start(out=wcat[:, k, 0:32], in_=w_attn[k * 128:(k + 1) * 128, :])
        nc.sync.dma_start(out=wcat[:, k, 32:96], in_=w_off[k * 128:(k + 1) * 128, :])
    wcat16 = keep.tile([128, 2, 96], F16)
    nc.scalar.copy(sv(wcat16, 0, [[1, 192]]), sv(wcat, 0, [[1, 192]]))
    bias96 = keep.tile([128, 96], F32)
    nc.sync.dma_start(out=bias96[:, 0:32], in_=_ap(b_attn, 0, [[0, 128], [1, 32]]))
    nc.sync.dma_start(out=bias96[:, 32:96], in_=_ap(b_off, 0, [[0, 128], [1, 64]]))
    ebt = keep.tile([128, L, HD], F32)
    nc.sync.dma_start(out=ebt[:], in_=_ap(embed_b, 0, [[0, 128], [1, L * HD]]))
    ebt16 = keep.tile([128, L, HD], F16)
    nc.scalar.copy(sv(ebt16, 0, [[1, L * HD]]), sv(ebt, 0, [[1, L * HD]]))
    c4 = keep.tile([128, 2 * 1024], F32)    # corner coefs, levels 0,1
    c4h = keep.tile([128, 2 * 1024], F16)   # fp16, x16 (pool-avg compensation)
    idxw = keep.tile([128, 2 * 2048], I16)  # wrapped dma_gather indices lv0,1
    permP = keep.tile([128, 128], F32)
    nc.sync.dma_start(out=permP[:], in_=io["permP"][:])
    lg_all = keep.tile([128, QB, 96], F32)
    refc = keep.tile([128, QB * 2], F32)
    nc.sync.dma_start(out=refc[:], in_=_ap(ref, 0, [[2, 128], [256, QB], [1, 2]]))
    xgt = keep.tile([128, 16], F32)
    nc.sync.dma_start(out=xgt[:], in_=_ap(io["xg"], 0, [[0, 128], [1, 16]]))
    zh = keep.tile([128, 128], F16)
    nc.vector.memset(zh[:], 0.0)
    febf2 = keep.tile([128, 2, HD], F16)
    febf3 = keep.tile([128, HD], F16)
    febf = {2: febf2, 3: febf3}  # fp16 femb tables for dense levels

    # ================= phase B: femb for all levels =================
    with ExitStack() as pb:
        fpool = pb.enter_context(tc.tile_pool(name="fpool", bufs=1))
        fsm = pb.enter_context(tc.tile_pool(name="fsm", bufs=2))
        psB = pb.enter_context(tc.tile_pool(name="psB", bufs=2, space="PSUM"))
        for lv, (H, W) in enumerate(LEVEL_HW):
            HW = H * W
            MT = (HW + 127) // 128
            fsb = fpool.tile([128, 2, HW], F32, tag="feat")
            fl = feats[lv].rearrange("c h w -> c (h w)")
            for k in range(2):
                nc.sync.dma_start(out=fsb[:, k, :], in_=fl[k * 128:(k + 1) * 128, :])
            fsb16 = fpool.tile([128, 2, HW], F16, tag="feat16")
            for k in range(2):
                nc.scalar.copy(fsb16[:, k, :], fsb[:, k, :])
            ew = fsm.tile([128, 2, HD], F32, tag="ew")
            for k in range(2):
                nc.sync.dma_start(out=ew[:, k, :],
                                  in_=embed_w[lv, k * 128:(k + 1) * 128, :])
            ew16 = fsm.tile([128, 2, HD], F16, tag="ew16")
            nc.scalar.copy(sv(ew16, 0, [[1, 2 * HD]]), sv(ew, 0, [[1, 2 * HD]]))
            if lv in DENSE_LV:
                fe16 = febf[lv]
            else:
                fe16 = fsm.tile([128, MT * HD], F16, tag="fe16")
            for m in range(MT):
                mp = min(128, HW - m * 128)
                psf = psB.tile([128, HD], F32, tag="psA")
                for k in range(2):
                    nc.tensor.matmul(
                        psf[:mp, :], lhsT=fsb16[:, k, m * 128:m * 128 + mp],
                        rhs=ew16[:, k, :], start=(k == 0), stop=(k == 1),
                    )
                if lv == 2:
                    nc.scalar.copy(fe16[:mp, m, :], psf[:mp, :])
                elif lv == 3:
                    nc.scalar.copy(fe16[:mp, :], psf[:mp, :])
                else:
                    nc.scalar.copy(fe16[:mp, m * HD:(m + 1) * HD], psf[:mp, :])
            if lv in DESC_LV:
                # femb4[r, k*32:(k+1)*32] = femb[r + sig], sig in (0, W, 1, W+1)
                f4 = femb4[lv]
                # zero the tail slivers the shifted stores leave uncovered
                # (never fetched: indices clamp to <= HW-W-2; disjoint from
                # the data stores so DMA completion order doesn't matter)
                nc.sync.dma_start(
                    out=_ap(f4, (HW - W) * 128 + 32, [[128, W], [1, 32]]),
                    in_=zh[0:W, 0:32])
                nc.sync.dma_start(
                    out=_ap(f4, (HW - 1) * 128 + 64, [[128, 1], [1, 32]]),
                    in_=zh[0:1, 0:32])
                nc.sync.dma_start(
                    out=_ap(f4, (HW - W - 1) * 128 + 96, [[128, W + 1], [1, 32]]),
                    in_=zh[0:W + 1, 0:32])
                for k, sig in enumerate((0, W, 1, W + 1)):
                    nc.sync.dma_start(
                        out=_ap(f4, k * 32, [[128, 128 - sig], [1, 32]]),
                        in_=fe16[sig:128, 0:32],
                    )
                    nc.sync.dma_start(
                        out=_ap(f4, (128 - sig) * 128 + k * 32,
                                [[128, 128], [16384, MT - 1], [1, 32]]),
                        in_=sv(fe16, HD, [[HD, MT - 1], [1, HD]]),
                    )

    # ================= phase C: logits for all blocks =================
    with ExitStack() as pc:
        xpool = pc.enter_context(tc.tile_pool(name="xpool", bufs=2))
        psT = pc.enter_context(tc.tile_pool(name="psT", bufs=2, space="PSUM"))
        psLg = pc.enter_context(tc.tile_pool(name="psLg", bufs=2, space="PSUM"))
        for lv in range(L):
            xlev = xpool.tile([128, BPL, 256], F32, tag="xlev")
            nc.sync.dma_start(
                out=xlev[:],
                in_=_ap(x, lv * P * C, [[256, 128], [128 * 256, BPL], [1, 256]]),
            )
            xh = xpool.tile([128, BPL, 256], F16, tag="xh")
            nc.scalar.copy(sv(xh, 0, [[1, BPL * 256]]), sv(xlev, 0, [[1, BPL * 256]]))
            for g in range(BPL):
                pt_ = psT.tile([128, 256], F16, tag="ptr")
                for k in range(2):
                    nc.tensor.transpose(pt_[:, k * 128:(k + 1) * 128],
                                        xh[:, g, k * 128:(k + 1) * 128], identH[:])
                xt = xpool.tile([128, 2, 128], F16, tag="xt")
                nc.scalar.copy(sv(xt, 0, [[1, 256]]), sv(pt_, 0, [[1, 256]]))
                lg = psLg.tile([128, 96], F32, tag="plg")
                for k in range(2):
                    nc.tensor.matmul(lg[:], lhsT=xt[:, k, :], rhs=wcat16[:, k, :],
                                     start=(k == 0), stop=(k == 1))
                nc.scalar.copy(lg_all[:, lv * BPL + g, :], lg[:])
            nc.vector.tensor_add(
                lg_all[:, lv * BPL:(lv + 1) * BPL, :],
                lg_all[:, lv * BPL:(lv + 1) * BPL, :],
                sv(bias96, 0, [[0, BPL], [1, 96]]))

    prep = ctx.enter_context(tc.tile_pool(name="prep", bufs=1))

    def softmax_pos(lv, W, scale16):
        """Common prep: attention softmax a_h and pixel positions T1."""
        g0 = lv * BPL
        kap = 0.5 * (W - 1)
        ea = prep.tile([128, 256], F32, tag="ea")
        nc.scalar.activation(
            ea[:], sv(lg_all, g0 * 96, [[96, BPL], [1, 32]]), AF.Exp)
        s2 = prep.tile([128, 128], F32, tag="s2")
        nc.vector.tensor_add(s2[:], sv(ea, 0, [[4, 64], [1, 2]]),
                             sv(ea, 2, [[4, 64], [1, 2]]))
        s1 = prep.tile([128, 64], F32, tag="s1")
        nc.vector.tensor_add(s1[:], sv(s2, 0, [[2, 64]]),
                             sv(s2, 1, [[2, 64]]))
        dinv = prep.tile([128, 64], F32, tag="dinv")
        nc.vector.reciprocal(dinv[:], s1[:])
        a_h = prep.tile([128, 256], F32, tag="a_h")
        nc.vector.tensor_mul(a_h[:], ea[:],
                             sv(dinv, 0, [[1, 64], [0, 4]]))
        T1 = prep.tile([128, 512], F32, tag="T1")
        nc.scalar.activation(
            T1[:], sv(lg_all, g0 * 96 + 32, [[96, BPL], [1, 64]]), AF.Tanh)
        nc.vector.tensor_add(T1[:], T1[:],
                             sv(refc, g0 * 2, [[2, BPL], [0, 32], [1, 2]]))
        nc.scalar.activation(T1[:], T1[:], AF.Copy, bias=kap, scale=kap)
        return a_h, T1

    # ============ phase D: prep + indices for descriptor levels ============
    pd_stack = ctx.enter_context(ExitStack())
    psP = pd_stack.enter_context(tc.tile_pool(name="psP", bufs=2, space="PSUM"))
    for lv in DESC_LV:
        H, W = LEVEL_HW[lv]
        g0 = lv * BPL
        a_h, T1 = softmax_pos(lv, W, None)

        T2 = prep.tile([128, 512], F32, tag="T2")
        nc.scalar.activation(T2[:], T1[:], AF.Copy, bias=RNE_M)
        nc.scalar.activation(T2[:], T2[:], AF.Copy, bias=-RNE_M)
        T3 = prep.tile([128, 512], F32, tag="T3")
        nc.vector.tensor_tensor(T3[:], T2[:], T1[:], OP.is_gt)
        nc.vector.tensor_tensor(T2[:], T2[:], T3[:], OP.subtract)   # x0f
        nc.vector.tensor_tensor(T3[:], T1[:], T2[:], OP.subtract)   # w1f
        nc.scalar.activation(T1[:], T3[:], AF.Copy, bias=1.0, scale=-1.0)
        T4 = prep.tile([128, 512], F32, tag="T4")  # xb
        nc.vector.tensor_scalar(T4[:], T2[:], 0.0, float(W - 2),
                                OP.max, OP.min)
        nc.vector.tensor_tensor(T2[:], T2[:], T4[:], OP.subtract)   # d
        T5 = prep.tile([128, 512], F32, tag="T5")  # e0 -> wB
        nc.vector.tensor_scalar(T5[:], T2[:], 0.0, None, OP.is_equal)
        T6 = prep.tile([128, 512], F32, tag="T6")  # em1
        nc.vector.tensor_scalar(T6[:], T2[:], -1.0, None, OP.is_equal)
        nc.vector.tensor_scalar(T2[:], T2[:], 1.0, None, OP.is_equal)
        T7 = prep.tile([128, 512], F32, tag="T7")  # wA
        nc.vector.tensor_tensor(T7[:], T1[:], T5[:], OP.mult)
        nc.vector.tensor_tensor(T6[:], T3[:], T6[:], OP.mult)
        nc.vector.tensor_add(T7[:], T7[:], T6[:])
        nc.vector.tensor_tensor(T5[:], T3[:], T5[:], OP.mult)
        nc.vector.tensor_tensor(T2[:], T1[:], T2[:], OP.mult)
        nc.vector.tensor_add(T5[:], T5[:], T2[:])

        fly = prep.tile([128, 256], F32, tag="fly")
        nc.vector.tensor_scalar_mul(fly[:], sv(T4, 1, [[2, 256]]), float(W))
        nc.vector.tensor_add(fly[:], fly[:], sv(T4, 0, [[2, 256]]))
        T2i = prep.tile([128, 2, 128], I16, tag="T2i")
        for j in range(2):
            pf = psP.tile([128, 128], F32, tag="ptrf")
            nc.tensor.matmul(pf[:], lhsT=fly[:, j * 128:(j + 1) * 128],
                             rhs=permP[:], start=True, stop=True)
            nc.vector.tensor_copy(T2i[:, j, :], pf[:])
        for j in range(2):
            for ql in range(16):
                nc.sync.dma_start(
                    out=sv(idxw[ql:ql + 1, :], lv * 2048 + j * 1024,
                           [[8, 128], [1, 8]]),
                    in_=T2i[:, j, ql * 8:(ql + 1) * 8],
                )
        for t0, tn in ((16, 16), (32, 32), (64, 64)):
            nc.sync.dma_start(
                out=idxw[t0:t0 + tn, lv * 2048:(lv + 1) * 2048],
                in_=idxw[0:tn, lv * 2048:(lv + 1) * 2048])

        wxa = prep.tile([128, 256], F32, tag="wxa")
        nc.vector.tensor_mul(wxa[:], sv(T7, 0, [[2, 256]]), a_h[:])
        wxb = prep.tile([128, 256], F32, tag="wxb")
        nc.vector.tensor_mul(wxb[:], sv(T5, 0, [[2, 256]]), a_h[:])
        for si, wx in ((0, wxa), (1, wxb)):
            for yi, wy in ((0, T7), (1, T5)):
                nc.vector.tensor_mul(
                    sv(c4, lv * 1024 + si * 2 + yi, [[4, 256]]),
                    wx[:],
                    sv(wy, 1, [[2, 256]]),
                )
        # fp16 corner weights
        nc.scalar.copy(c4h[:, lv * 1024:(lv + 1) * 1024],
                       c4[:, lv * 1024:(lv + 1) * 1024])

    # ============ phase E: dense-hat levels (2, 3) ============
    pd_stack.close()  # release phase-D PSUM banks
    dn = ctx.enter_context(tc.tile_pool(name="dn", bufs=1))
    dn2 = ctx.enter_context(tc.tile_pool(name="dn2", bufs=2))
    dntmp = ctx.enter_context(tc.tile_pool(name="dntmp", bufs=1))
    psD = ctx.enter_context(tc.tile_pool(name="psD", bufs=2, space="PSUM"))
    psO = ctx.enter_context(tc.tile_pool(name="psO", bufs=2, space="PSUM"))
    opool = ctx.enter_context(tc.tile_pool(name="opool", bufs=2))

    for lv in (() if SKIP_DENSE else DENSE_LV):
        H, W = LEVEL_HW[lv]
        WW = H * W
        a_h, T1 = softmax_pos(lv, W, None)
        a_h16 = prep.tile([128, 256], F16, tag="a_h16")
        nc.scalar.copy(a_h16[:], a_h[:])

        # hat weights over the grid: relu(1 - |grid - pos|), fp32 -> fp16
        hx16 = dn.tile([128, 256, 16], F16, tag="hx16")
        hy16 = dn.tile([128, 256, 16], F16, tag="hy16")
        dxt = dn.tile([128, 256, 16], F16, tag="dxt")
        for ax, ht in ((0, hx16), (1, hy16)):
            nc.vector.tensor_tensor(
                sv(dxt, 0, [[W, 256], [1, W]]),
                sv(xgt, 0, [[0, 256], [1, W]]),
                sv(T1, ax, [[2, 256], [0, W]]),
                OP.subtract)
            nc.scalar.activation(sv(dxt, 0, [[1, 256 * W]]),
                                 sv(dxt, 0, [[1, 256 * W]]), AF.Abs)
            nc.scalar.activation(sv(ht, 0, [[1, 256 * W]]),
                                 sv(dxt, 0, [[1, 256 * W]]),
                                 AF.Relu, bias=1.0, scale=-1.0)
        # fold attn (x4) into hy
        nc.vector.tensor_mul(sv(hy16, 0, [[W, 256], [1, W]]),
                             sv(hy16, 0, [[W, 256], [1, W]]),
                             sv(a_h16, 0, [[1, 256], [0, W]]))

        for qb in range(BPL):
            tmpt = dntmp.tile([128, 4, 8 * WW], F16, tag=f"tmp{lv}")
            base = qb * 32 * W
            for ns in range(NS):
                nc.vector.tensor_mul(
                    sv(tmpt, ns * 8 * WW, [[WW, 8], [W, W], [1, W]]),
                    sv(hy16, base + ns * W, [[4 * W, 8], [1, W], [0, W]]),
                    sv(hx16, base + ns * W, [[4 * W, 8], [0, W], [1, W]]),
                )
            At = dn2.tile([128, 8 * WW], F16, tag=f"At{lv}")
            nc.vector.tensor_add(sv(tmpt, 0, [[1, 8 * WW]]),
                                 sv(tmpt, 0, [[1, 8 * WW]]),
                                 sv(tmpt, 8 * WW, [[1, 8 * WW]]))
            nc.vector.tensor_add(sv(tmpt, 2 * 8 * WW, [[1, 8 * WW]]),
                                 sv(tmpt, 2 * 8 * WW, [[1, 8 * WW]]),
                                 sv(tmpt, 3 * 8 * WW, [[1, 8 * WW]]))
            nc.vector.tensor_add(sv(At, 0, [[1, 8 * WW]]),
                                 sv(tmpt, 0, [[1, 8 * WW]]),
                                 sv(tmpt, 2 * 8 * WW, [[1, 8 * WW]]))
            pso = psO.tile([128, 256], F32, tag="psout")
            for hh in range(2):
                if lv == 2:
                    # 4 heads of A^T chunks transposed into one PSUM bank
                    pst = psD.tile([128, 4, 2, 128], F16, tag="pst2")
                    for h4 in range(4):
                        h = hh * 4 + h4
                        for cch in range(2):
                            nc.tensor.transpose(
                                pst[:, h4, cch, :],
                                At[:, h * WW + cch * 128: h * WW + (cch + 1) * 128],
                                identH[:])
                    Ah = dn2.tile([128, 4, 2, 128], F16, tag="Ah2")
                    nc.scalar.copy(sv(Ah, 0, [[1, 1024]]), sv(pst, 0, [[1, 1024]]))
                    for h4 in range(4):
                        h = hh * 4 + h4
                        for cch in range(2):
                            nc.tensor.matmul(
                                pso[:, h * 32:(h + 1) * 32],
                                lhsT=Ah[:, h4, cch, :],
                                rhs=febf[2][:, cch, :],
                                start=(cch == 0), stop=(cch == 1))
                else:
                    pst = psD.tile([64, 4, 128], F16, tag="pst3")
                    for h4 in range(4):
                        h = hh * 4 + h4
                        nc.tensor.transpose(pst[:, h4, :],
                                            At[:, h * WW:(h + 1) * WW],
                                            identH[:])
                    Ah = dn2.tile([64, 4, 128], F16, tag="Ah3")
                    nc.scalar.copy(sv(Ah, 0, [[1, 512]]), sv(pst, 0, [[1, 512]]))
                    for h4 in range(4):
                        h = hh * 4 + h4
                        nc.tensor.matmul(
                            pso[:, h * 32:(h + 1) * 32],
                            lhsT=Ah[:, h4, :], rhs=febf[3][0:64, :],
                            start=True, stop=True)
            ob = opool.tile([128, 256], F32, tag="ob")
            nc.vector.tensor_add(ob[:], pso[:],
                                 sv(ebt, lv * HD, [[0, 8], [1, 32]]))
            nc.sync.dma_start(
                out=_ap(out, (lv * BPL + qb) * 128 * 256, [[256, 128], [1, 256]]),
                in_=ob[:],
            )

    # ============ phase F: gathers + combines for levels 0,1 ============
    gpool = ctx.enter_context(tc.tile_pool(name="gpool", bufs=3))
    cpool = ctx.enter_context(tc.tile_pool(name="cpool", bufs=2))
    for lv in (() if SKIP_GATHER else DESC_LV):
        H, W = LEVEL_HW[lv]
        HW = H * W
        for g2 in range(BPL // 2):  # 2-block supertiles
            blk0 = lv * BPL + 2 * g2
            gb = gpool.tile([128, 2, 32, 128], F16, tag="gb")
            for bi in range(2):
                blk = blk0 + bi
                for hf in range(4):
                    nc.gpsimd.dma_gather(
                        gb[:, bi, hf * 8:(hf + 1) * 8, :],
                        _ap(femb4[lv], 0, [[128, HW], [1, 128]]),
                        idxw[:, blk * 256 + hf * 64: blk * 256 + (hf + 1) * 64],
                        1024,
                        1024,
                        128,
                        elem_step=128,
                        queue_num=hf,
                    )
            # (block, sample) merge into one uniform-stride dim keeps all
            # combine ops within the 3-free-dim ISA limit
            nc.vector.tensor_mul(
                sv(gb, 0, [[128, 64], [32, 4], [1, 32]]),
                sv(gb, 0, [[128, 64], [32, 4], [1, 32]]),
                sv(c4h, lv * 1024 + 2 * g2 * 128, [[4, 64], [1, 4], [0, 32]]),
            )
            # reduction tree over (corner, sample): 2x4096 -> 2x256
            ta = cpool.tile([128, 2, 2048], F16, tag="ta")
            nc.vector.tensor_add(
                sv(ta, 0, [[64, 64], [1, 64]]),
                sv(gb, 0, [[128, 64], [1, 64]]),
                sv(gb, 64, [[128, 64], [1, 64]]))
            tb = cpool.tile([128, 2, 1024], F16, tag="tb")
            nc.vector.tensor_add(
                sv(tb, 0, [[32, 64], [1, 32]]),
                sv(ta, 0, [[64, 64], [1, 32]]),
                sv(ta, 32, [[64, 64], [1, 32]]))
            tc2 = cpool.tile([128, 2, 512], F16, tag="tc2")
            nc.vector.tensor_add(
                sv(tc2, 0, [[64, 16], [1, 64]]),
                sv(tb, 0, [[128, 16], [1, 64]]),
                sv(tb, 64, [[128, 16], [1, 64]]))
            ob16 = cpool.tile([128, 2, 256], F16, tag="ob16")
            nc.vector.tensor_add(
                sv(ob16, 0, [[32, 16], [1, 32]]),
                sv(tc2, 0, [[64, 16], [1, 32]]),
                sv(tc2, 32, [[64, 16], [1, 32]]))
            nc.vector.tensor_add(
                sv(ob16, 0, [[32, 16], [1, 32]]),
                sv(ob16, 0, [[32, 16], [1, 32]]),
                sv(ebt16, lv * HD, [[0, 16], [1, 32]]))
            ob = opool.tile([128, 2, 256], F32, tag="obg")
            nc.scalar.copy(sv(ob, 0, [[1, 512]]), sv(ob16, 0, [[1, 512]]))
            nc.sync.dma_start(
                out=_ap(out, blk0 * 128 * 256,
                        [[256, 128], [128 * 256, 2], [1, 256]]),
                in_=ob[:],
            )


def build_program():
    nc = bacc.Bacc("TRN2", target_bir_lowering=False, debug=False,
                   num_swdge_queues=4)
    io = {}
    io["x"] = nc.dram_tensor("x", [L, P, C], F32, kind="ExternalInput").ap()
    io["ref"] = nc.dram_tensor("ref", [L, P, 2], F32, kind="ExternalInput").ap()
    for i, (H, W) in enumerate(LEVEL_HW):
        io[f"feat{i}"] = nc.dram_tensor(f"feat{i}", [C, H, W], F32,
                                        kind="ExternalInput").ap()
    io["w_attn"] = nc.dram_tensor("w_attn", [C, NH * NS], F32,
                                  kind="ExternalInput").ap()
    io["b_attn"] = nc.dram_tensor("b_attn", [NH * NS], F32,
                                  kind="ExternalInput").ap()
    io["w_off"] = nc.dram_tensor("w_off", [C, 2 * NH * NS], F32,
                                 kind="ExternalInput").ap()
    io["b_off"] = nc.dram_tensor("b_off", [2 * NH * NS], F32,
                                 kind="ExternalInput").ap()
    io["embed_w"] = nc.dram_tensor("embed_w", [L, C, HD], F32,
                                   kind="ExternalInput").ap()
    io["embed_b"] = nc.dram_tensor("embed_b", [L, HD], F32,
                                   kind="ExternalInput").ap()
    io["permP"] = nc.dram_tensor("permP", [128, 128], F32,
                                 kind="ExternalInput").ap()
    io["xg"] = nc.dram_tensor("xg", [16], F32, kind="ExternalInput").ap()
    io["out"] = nc.dram_tensor("out", [L, P, NH * HD], F32,
                               kind="ExternalOutput").ap()
    io["femb4"] = {
        lv: nc.dram_tensor(f"femb4_{lv}", [LEVEL_HW[lv][0] * LEVEL_HW[lv][1], 128],
                           F16, kind="Internal").ap()
        for lv in DESC_LV
    }
    with tile.TileContext(nc) as tc:
        with ExitStack() as ctx:
            emit_kernel(ctx, tc, io)
    nc.compile()
    return nc


_prog = None


def kernel(**inputs):
    global _prog
    if _prog is None:
        _prog = build_program()
    nc = _prog
    res = run_bass_kernel_spmd(nc, _in_maps(inputs), list(range(B)))
    out = np.stack([res.results[i]["out"] for i in range(B)], axis=0)
    return out.reshape(B, L, P, NH * HD)


def _perm_matrix():
    p = np.zeros((128, 128), np.float32)
    for n in range(128):
        p[(n % 8) * 16 + n // 8, n] = 1.0
    return p


def _in_maps(inputs):
    keys = ["x", "ref", "feat0", "feat1", "feat2", "feat3",
            "w_attn", "b_attn", "w_off", "b_off", "embed_w", "embed_b"]
    per_batch = {"x", "ref", "feat0", "feat1", "feat2", "feat3"}
    pm = _perm_matrix()
    xg = np.arange(16, dtype=np.float32)
    maps = []
    for b in range(B):
        m = {"permP": pm, "xg": xg}
        for kk in keys:
            v = np.ascontiguousarray(np.asarray(inputs[kk], dtype=np.float32))
            m[kk] = v[b] if kk in per_batch else v
        maps.append(m)
    return maps


def profile(inputs):
    """Run with tracing; returns HW exec time in ns (or None if unavailable)."""
    global _prog
    if _prog is None:
        _prog = build_program()
    res = run_bass_kernel_spmd(_prog, _in_maps(inputs), list(range(B)), trace=True)
    return res.exec_time_ns


if __name__ == "__main__":
    build_program()
    print("build ok")


# revision 39
# speedup vs baseline: 1.6202x; 1.0740x over previous
"""Trainium2 Bass kernel for nn_DeformableBlock (deformable attention block).

Per core = one batch element (data-parallel over batch). Two gather paths:

Levels 0,1 (64x64 / 32x32): descriptor gather, pipelined per level so the
  serial SWDGE descriptor generation (the critical resource) starts ~40us in.
  femb4[r] = [femb[r], femb[r+W], femb[r+1], femb[r+W+1]] packed fp16 256B
  rows: ONE dma_gather descriptor fetches all 4 bilinear corners of a point.
  fp16 combine: corner weights pre-expanded per channel on the scalar engine
  so the big multiply runs contiguous fp16 at 2 elem/cycle on DVE, then an
  in-place reduction tree (reads lead writes) inside the gather tile.

Levels 2,3 (16x16 / 8x8): no gather.
  Bilinear+attention weights evaluated DENSELY over the small grid as a
  separable hat product A[q,(Y,X)] = sum_s attn_s*hat(Y-fy_s)*hat(X-fx_s)
  (exact vs zero-padding bilinear), then out = A @ femb on the PE.
  A-builds are interleaved with the combines to fill DVE gaps while the
  Pool engine grinds through descriptor generation.

fp16 everywhere on the value path (8x finer rounding than bf16); fp32 for
positions/softmax. Inputs are cast f32->fp16 during the DMA (SWDGE).
"""

import os
import sys

for _p in ("/opt/trn_rl_repo",):
    if _p not in sys.path:
        sys.path.insert(0, _p)

import numpy as np
from contextlib import ExitStack

import concourse.bass as bass
import concourse.bacc as bacc
import concourse.tile as tile
from concourse import mybir
from concourse.bass import AP
from concourse.bass_utils import run_bass_kernel_spmd
from concourse.masks import make_identity

F32 = mybir.dt.float32
F16 = mybir.dt.float16
I16 = mybir.dt.int16
AF = mybir.ActivationFunctionType
OP = mybir.AluOpType

B, L, P, C = 8, 4, 1024, 256
NH, NS, HD = 8, 4, 32
LEVEL_HW = [(64, 64), (32, 32), (16, 16), (8, 8)]
NQ = L * P
QB = NQ // 128      # 32 query blocks of 128
BPL = QB // L       # 8 blocks per level
RNE_M = 12582912.0  # 1.5*2^23; f+M lands in [2^23,2^24) where ulp==1
DESC_LV = (0, 1)
DENSE_LV = (2, 3)


def _ap(t, offset, dims):
    """Raw AP on a DRAM tensor: offset and strides in flat elements."""
    return AP(tensor=t.tensor if isinstance(t, AP) else t, offset=offset,
              ap=[list(d) for d in dims])


def sv(t: AP, off: int, dims):
    """Strided free-dim view of an SBUF tile: keeps the partition dim,
    offsets `off` elements into each partition's free space."""
    base = t[:] if not isinstance(t, AP) else t
    pstride, nparts = base.ap[0]
    return AP(tensor=base.tensor, offset=base.offset + off,
              ap=[[pstride, nparts]] + [list(d) for d in dims])


def emit_kernel(ctx: ExitStack, tc: tile.TileContext, io: dict):
    nc = tc.nc
    x, ref = io["x"], io["ref"]
    feats = [io[f"feat{i}"] for i in range(L)]
    embed_w, embed_b = io["embed_w"], io["embed_b"]
    out = io["out"]
    femb4 = io["femb4"]

    keep = ctx.enter_context(tc.tile_pool(name="keep", bufs=1))

    # ---- long-lived constants (f32->f16 casts ride the SWDGE DMA) ----
    identH = keep.tile([128, 128], F16)
    make_identity(nc, identH)
    wcat16 = keep.tile([128, 2, 96], F16)
    for k in range(2):
        nc.gpsimd.dma_start(out=wcat16[:, k, 0:32],
                            in_=io["w_attn"][k * 128:(k + 1) * 128, :])
        nc.gpsimd.dma_start(out=wcat16[:, k, 32:96],
                            in_=io["w_off"][k * 128:(k + 1) * 128, :])
    bias96 = keep.tile([128, 96], F32)
    nc.sync.dma_start(out=bias96[:, 0:32], in_=_ap(io["b_attn"], 0, [[0, 128], [1, 32]]))
    nc.sync.dma_start(out=bias96[:, 32:96], in_=_ap(io["b_off"], 0, [[0, 128], [1, 64]]))
    ebt = keep.tile([128, L, HD], F32)
    nc.sync.dma_start(out=ebt[:], in_=_ap(embed_b, 0, [[0, 128], [1, L * HD]]))
    ebt16 = keep.tile([128, L, HD], F16)
    nc.scalar.copy(sv(ebt16, 0, [[1, L * HD]]), sv(ebt, 0, [[1, L * HD]]))
    c4h = keep.tile([128, 2 * 1024], F16)   # fp16 corner weights (lv0,1)
    idxw = keep.tile([128, 2 * 2048], I16)
    permP = keep.tile([128, 128], F32)
    nc.sync.dma_start(out=permP[:], in_=io["permP"][:])
    lg_all = keep.tile([128, QB, 96], F32)
    refc = keep.tile([128, QB * 2], F32)
    nc.sync.dma_start(out=refc[:], in_=_ap(ref, 0, [[2, 128], [256, QB], [1, 2]]))
    xgt = keep.tile([128, 16], F32)
    nc.sync.dma_start(out=xgt[:], in_=_ap(io["xg"], 0, [[0, 128], [1, 16]]))
    zh = keep.tile([128, 128], F16)
    nc.vector.memset(zh[:], 0.0)
    febf2 = keep.tile([128, 2, HD], F16)
    febf3 = keep.tile([128, HD], F16)
    febf = {2: febf2, 3: febf3}

    # pools (everything coexists; gathers overlap the dense phase)
    fpool = ctx.enter_context(tc.tile_pool(name="fpool", bufs=1))
    fsm = ctx.enter_context(tc.tile_pool(name="fsm", bufs=2))
    xpool = ctx.enter_context(tc.tile_pool(name="xpool", bufs=2))
    prep = ctx.enter_context(tc.tile_pool(name="prep", bufs=1))
    dn = ctx.enter_context(tc.tile_pool(name="dn", bufs=1))
    dn2 = ctx.enter_context(tc.tile_pool(name="dn2", bufs=2))
    dntmp = ctx.enter_context(tc.tile_pool(name="dntmp", bufs=1))
    gpool = ctx.enter_context(tc.tile_pool(name="gpool", bufs=2))
    cpool = ctx.enter_context(tc.tile_pool(name="cpool", bufs=2))
    cxp = ctx.enter_context(tc.tile_pool(name="cxp", bufs=1))
    opool = ctx.enter_context(tc.tile_pool(name="opool", bufs=2))
    psA = ctx.enter_context(tc.tile_pool(name="psA", bufs=1, space="PSUM"))
    psLg = ctx.enter_context(tc.tile_pool(name="psLg", bufs=1, space="PSUM"))
    psT = ctx.enter_context(tc.tile_pool(name="psT", bufs=1, space="PSUM"))
    psP = ctx.enter_context(tc.tile_pool(name="psP", bufs=1, space="PSUM"))
    psD = ctx.enter_context(tc.tile_pool(name="psD", bufs=2, space="PSUM"))
    psO = ctx.enter_context(tc.tile_pool(name="psO", bufs=1, space="PSUM"))

    # ---------------- phase helpers ----------------

    def emit_femb(lv):
        """Project feat_lv through embed_w[lv] -> fp16 femb; pack femb4 for
        descriptor levels, keep SBUF table for dense levels."""
        H, W = LEVEL_HW[lv]
        HW = H * W
        MT = (HW + 127) // 128
        fsb16 = fpool.tile([128, 2, 4096], F16, tag="feat16")
        fl = feats[lv].rearrange("c h w -> c (h w)")
        for k in range(2):
            nc.gpsimd.dma_start(out=fsb16[:, k, 0:HW], in_=fl[k * 128:(k + 1) * 128, :])
        ew16 = fsm.tile([128, 2, HD], F16, tag="ew16")
        for k in range(2):
            nc.gpsimd.dma_start(out=ew16[:, k, :],
                                in_=embed_w[lv, k * 128:(k + 1) * 128, :])
        if lv in DENSE_LV:
            fe16 = febf[lv]
        else:
            fe16 = fsm.tile([128, 32 * HD], F16, tag="fe16")
        for m in range(MT):
            mp = min(128, HW - m * 128)
            psf = psA.tile([128, HD], F32, tag="psf")
            for k in range(2):
                nc.tensor.matmul(
                    psf[:mp, :], lhsT=fsb16[:, k, m * 128:m * 128 + mp],
                    rhs=ew16[:, k, :], start=(k == 0), stop=(k == 1),
                )
            if lv == 2:
                nc.scalar.copy(fe16[:mp, m, :], psf[:mp, :])
            elif lv == 3:
                nc.scalar.copy(fe16[:mp, :], psf[:mp, :])
            else:
                nc.scalar.copy(fe16[:mp, m * HD:(m + 1) * HD], psf[:mp, :])
        if lv in DESC_LV:
            f4 = femb4[lv]
            # zero the tail slivers the shifted stores leave uncovered
            nc.sync.dma_start(
                out=_ap(f4, (HW - W) * 128 + 32, [[128, W], [1, 32]]),
                in_=zh[0:W, 0:32])
            nc.sync.dma_start(
                out=_ap(f4, (HW - 1) * 128 + 64, [[128, 1], [1, 32]]),
                in_=zh[0:1, 0:32])
            nc.sync.dma_start(
                out=_ap(f4, (HW - W - 1) * 128 + 96, [[128, W + 1], [1, 32]]),
                in_=zh[0:W + 1, 0:32])
            # femb4[r, k*32:(k+1)*32] = femb[r + sig]
            for k, sig in enumerate((0, W, 1, W + 1)):
                nc.sync.dma_start(
                    out=_ap(f4, k * 32, [[128, 128 - sig], [1, 32]]),
                    in_=fe16[sig:128, 0:32],
                )
                nc.sync.dma_start(
                    out=_ap(f4, (128 - sig) * 128 + k * 32,
                            [[128, 128], [16384, MT - 1], [1, 32]]),
                    in_=sv(fe16, HD, [[HD, MT - 1], [1, HD]]),
                )

    def emit_logits(lv):
        """x[lv] -> fp16 (cast in DMA) -> transposed -> attn|off logits."""
        xh = xpool.tile([128, BPL, 256], F16, tag="xh")
        nc.gpsimd.dma_start(
            out=xh[:],
            in_=_ap(x, lv * P * C, [[256, 128], [128 * 256, BPL], [1, 256]]),
        )
        for g2 in range(BPL // 2):
            pt_ = psT.tile([128, 2, 256], F16, tag="ptr")
            for gi in range(2):
                g = 2 * g2 + gi
                for k in range(2):
                    nc.tensor.transpose(pt_[:, gi, k * 128:(k + 1) * 128],
                                        xh[:, g, k * 128:(k + 1) * 128], identH[:])
            xt = xpool.tile([128, 2, 2, 128], F16, tag="xt")
            nc.scalar.copy(sv(xt, 0, [[1, 512]]), sv(pt_, 0, [[1, 512]]))
            lg = psLg.tile([128, 2, 96], F32, tag="plg")
            for gi in range(2):
                for k in range(2):
                    nc.tensor.matmul(lg[:, gi, :], lhsT=xt[:, gi, k, :],
                                     rhs=wcat16[:, k, :],
                                     start=(k == 0), stop=(k == 1))
            nc.scalar.copy(
                sv(lg_all, (lv * BPL + 2 * g2) * 96, [[1, 192]]),
                sv(lg, 0, [[1, 192]]))
        nc.vector.tensor_add(
            lg_all[:, lv * BPL:(lv + 1) * BPL, :],
            lg_all[:, lv * BPL:(lv + 1) * BPL, :],
            sv(bias96, 0, [[0, BPL], [1, 96]]))

    def softmax_pos(lv, W):
        """Attention softmax a_h and pixel positions T1 (both f32)."""
        g0 = lv * BPL
        kap = 0.5 * (W - 1)
        ea = prep.tile([128, 256], F32, tag="ea")
        nc.scalar.activation(
            ea[:], sv(lg_all, g0 * 96, [[96, BPL], [1, 32]]), AF.Exp)
        s2 = prep.tile([128, 128], F32, tag="s2")
        nc.vector.tensor_add(s2[:], sv(ea, 0, [[4, 64], [1, 2]]),
                             sv(ea, 2, [[4, 64], [1, 2]]))
        s1 = prep.tile([128, 64], F32, tag="s1")
        nc.vector.tensor_add(s1[:], sv(s2, 0, [[2, 64]]),
                             sv(s2, 1, [[2, 64]]))
        dinv = prep.tile([128, 64], F32, tag="dinv")
        nc.vector.reciprocal(dinv[:], s1[:])
        a_h = prep.tile([128, 256], F32, tag="a_h")
        nc.vector.tensor_mul(a_h[:], ea[:],
                             sv(dinv, 0, [[1, 64], [0, 4]]))
        T1 = prep.tile([128, 512], F32, tag="T1")
        nc.scalar.activation(
            T1[:], sv(lg_all, g0 * 96 + 32, [[96, BPL], [1, 64]]), AF.Tanh)
        nc.vector.tensor_add(T1[:], T1[:],
                             sv(refc, g0 * 2, [[2, BPL], [0, 32], [1, 2]]))
        nc.scalar.activation(T1[:], T1[:], AF.Copy, bias=kap, scale=kap)
        return a_h, T1

    def emit_prep_desc(lv):
        """Floor/clamp/corner-weight/index machinery for a descriptor level."""
        H, W = LEVEL_HW[lv]
        a_h, T1 = softmax_pos(lv, W)
        T2 = prep.tile([128, 512], F32, tag="T2")
        nc.scalar.activation(T2[:], T1[:], AF.Copy, bias=RNE_M)
        nc.scalar.activation(T2[:], T2[:], AF.Copy, bias=-RNE_M)
        T3 = prep.tile([128, 512], F32, tag="T3")
        nc.vector.tensor_tensor(T3[:], T2[:], T1[:], OP.is_gt)
        nc.vector.tensor_tensor(T2[:], T2[:], T3[:], OP.subtract)   # x0f
        nc.vector.tensor_tensor(T3[:], T1[:], T2[:], OP.subtract)   # w1f
        nc.scalar.activation(T1[:], T3[:], AF.Copy, bias=1.0, scale=-1.0)
        T4 = prep.tile([128, 512], F32, tag="T4")  # xb
        nc.vector.tensor_scalar(T4[:], T2[:], 0.0, float(W - 2), OP.max, OP.min)
        nc.vector.tensor_tensor(T2[:], T2[:], T4[:], OP.subtract)   # d
        T5 = prep.tile([128, 512], F32, tag="T5")  # e0 -> wB
        nc.vector.tensor_scalar(T5[:], T2[:], 0.0, None, OP.is_equal)
        T6 = prep.tile([128, 512], F32, tag="T6")  # em1
        nc.vector.tensor_scalar(T6[:], T2[:], -1.0, None, OP.is_equal)
        nc.vector.tensor_scalar(T2[:], T2[:], 1.0, None, OP.is_equal)
        T7 = prep.tile([128, 512], F32, tag="T7")  # wA
        nc.vector.tensor_tensor(T7[:], T1[:], T5[:], OP.mult)
        nc.vector.tensor_tensor(T6[:], T3[:], T6[:], OP.mult)
        nc.vector.tensor_add(T7[:], T7[:], T6[:])
        nc.vector.tensor_tensor(T5[:], T3[:], T5[:], OP.mult)
        nc.vector.tensor_tensor(T2[:], T1[:], T2[:], OP.mult)
        nc.vector.tensor_add(T5[:], T5[:], T2[:])

        fly = prep.tile([128, 256], F32, tag="fly")
        nc.vector.tensor_scalar_mul(fly[:], sv(T4, 1, [[2, 256]]), float(W))
        nc.vector.tensor_add(fly[:], fly[:], sv(T4, 0, [[2, 256]]))
        T2i = prep.tile([128, 2, 128], I16, tag="T2i")
        for j in range(2):
            pf = psP.tile([128, 128], F32, tag="ptrf")
            nc.tensor.matmul(pf[:], lhsT=fly[:, j * 128:(j + 1) * 128],
                             rhs=permP[:], start=True, stop=True)
            nc.vector.tensor_copy(T2i[:, j, :], pf[:])
        for j in range(2):
            for ql in range(16):
                nc.sync.dma_start(
                    out=sv(idxw[ql:ql + 1, :], lv * 2048 + j * 1024,
                           [[8, 128], [1, 8]]),
                    in_=T2i[:, j, ql * 8:(ql + 1) * 8],
                )
        for t0, tn in ((16, 16), (32, 32), (64, 64)):
            nc.sync.dma_start(
                out=idxw[t0:t0 + tn, lv * 2048:(lv + 1) * 2048],
                in_=idxw[0:tn, lv * 2048:(lv + 1) * 2048])

        wxa = prep.tile([128, 256], F32, tag="wxa")
        nc.vector.tensor_mul(wxa[:], sv(T7, 0, [[2, 256]]), a_h[:])
        wxb = prep.tile([128, 256], F32, tag="wxb")
        nc.vector.tensor_mul(wxb[:], sv(T5, 0, [[2, 256]]), a_h[:])
        for si, wx in ((0, wxa), (1, wxb)):
            for yi, wy in ((0, T7), (1, T5)):
                nc.vector.tensor_mul(
                    sv(c4h, lv * 1024 + si * 2 + yi, [[4, 256]]),
                    wx[:],
                    sv(wy, 1, [[2, 256]]),
                )

    def emit_gathers(lv, g2):
        """One 2-block supertile of gathers; returns the gather tile."""
        H, W = LEVEL_HW[lv]
        HW = H * W
        blk0 = lv * BPL + 2 * g2
        gb = gpool.tile([128, 2, 32, 128], F16, tag="gb")
        for bi in range(2):
            blk = blk0 + bi
            for hf in range(4):
                nc.gpsimd.dma_gather(
                    gb[:, bi, hf * 8:(hf + 1) * 8, :],
                    _ap(femb4[lv], 0, [[128, HW], [1, 128]]),
                    idxw[:, blk * 256 + hf * 64: blk * 256 + (hf + 1) * 64],
                    1024, 1024, 128,
                    elem_step=128,
                    queue_num=hf,
                )
        return gb

    def emit_combine(lv, g2, gb):
        """Weighted 4-corner+sample reduction for one supertile."""
        blk0 = lv * BPL + 2 * g2
        # expand corner weights per-channel on ACT so the multiply is a
        # contiguous fp16 tensor_tensor (2 elem/cycle on DVE)
        c4x = cxp.tile([128, 8192], F16, tag="c4x")
        nc.scalar.copy(
            sv(c4x, 0, [[1, 8192]]),
            sv(c4h, lv * 1024 + 2 * g2 * 128, [[4, 64], [1, 4], [0, 32]]))
        nc.vector.tensor_mul(sv(gb, 0, [[1, 8192]]),
                             sv(gb, 0, [[1, 8192]]),
                             sv(c4x, 0, [[1, 8192]]))
        # in-place reduction tree (reads lead writes), 2x4096 -> 2x256
        nc.vector.tensor_add(
            sv(gb, 0, [[64, 64], [1, 64]]),
            sv(gb, 0, [[128, 64], [1, 64]]),
            sv(gb, 64, [[128, 64], [1, 64]]))
        nc.vector.tensor_add(
            sv(gb, 4096, [[32, 64], [1, 32]]),
            sv(gb, 0, [[64, 64], [1, 32]]),
            sv(gb, 32, [[64, 64], [1, 32]]))
        nc.vector.tensor_add(
            sv(gb, 6144, [[64, 16], [1, 64]]),
            sv(gb, 4096, [[128, 16], [1, 64]]),
            sv(gb, 4096 + 64, [[128, 16], [1, 64]]))
        ob16 = cpool.tile([128, 2, 256], F16, tag="ob16")
        nc.vector.tensor_add(
            sv(ob16, 0, [[32, 16], [1, 32]]),
            sv(gb, 6144, [[64, 16], [1, 32]]),
            sv(gb, 6144 + 32, [[64, 16], [1, 32]]))
        nc.vector.tensor_add(
            sv(ob16, 0, [[32, 16], [1, 32]]),
            sv(ob16, 0, [[32, 16], [1, 32]]),
            sv(ebt16, lv * HD, [[0, 16], [1, 32]]))
        ob = opool.tile([128, 2, 256], F32, tag="obg")
        nc.scalar.copy(sv(ob, 0, [[1, 512]]), sv(ob16, 0, [[1, 512]]))
        nc.sync.dma_start(
            out=_ap(out, blk0 * 128 * 256,
                    [[256, 128], [128 * 256, 2], [1, 256]]),
            in_=ob[:],
        )

    def emit_hats(lv):
        """Dense bilinear hat weights over the grid + attn fold (fp16)."""
        H, W = LEVEL_HW[lv]
        a_h, T1 = softmax_pos(lv, W)
        a_h16 = prep.tile([128, 256], F16, tag="a_h16")
        nc.scalar.copy(a_h16[:], a_h[:])
        hx16 = dn.tile([128, 256, W], F16, tag=f"hx{lv}")
        hy16 = dn.tile([128, 256, W], F16, tag=f"hy{lv}")
        dxt = dn.tile([128, 256, 16], F16, tag="dxt")
        for ax, ht in ((0, hx16), (1, hy16)):
            nc.vector.tensor_tensor(
                sv(dxt, 0, [[W, 256], [1, W]]),
                sv(xgt, 0, [[0, 256], [1, W]]),
                sv(T1, ax, [[2, 256], [0, W]]),
                OP.subtract)
            nc.scalar.activation(sv(dxt, 0, [[1, 256 * W]]),
                                 sv(dxt, 0, [[1, 256 * W]]), AF.Abs)
            nc.scalar.activation(sv(ht, 0, [[1, 256 * W]]),
                                 sv(dxt, 0, [[1, 256 * W]]),
                                 AF.Relu, bias=1.0, scale=-1.0)
        nc.vector.tensor_mul(sv(hy16, 0, [[W, 256], [1, W]]),
                             sv(hy16, 0, [[W, 256], [1, W]]),
                             sv(a_h16, 0, [[1, 256], [0, W]]))
        return hx16, hy16

    def emit_dense_qb(lv, qb, hx16, hy16):
        """A-build + PE gather-matmul + store for one query block."""
        H, W = LEVEL_HW[lv]
        WW = H * W
        tmpt = dntmp.tile([128, 4, 8 * 256], F16, tag="tmp")
        base = qb * 32 * W
        for ns in range(NS):
            nc.vector.tensor_mul(
                sv(tmpt, ns * 8 * WW, [[WW, 8], [W, W], [1, W]]),
                sv(hy16, base + ns * W, [[4 * W, 8], [1, W], [0, W]]),
                sv(hx16, base + ns * W, [[4 * W, 8], [0, W], [1, W]]),
            )
        At = dn2.tile([128, 8 * 256], F16, tag="At")
        nc.vector.tensor_add(sv(tmpt, 0, [[1, 8 * WW]]),
                             sv(tmpt, 0, [[1, 8 * WW]]),
                             sv(tmpt, 8 * WW, [[1, 8 * WW]]))
        nc.vector.tensor_add(sv(tmpt, 2 * 8 * WW, [[1, 8 * WW]]),
                             sv(tmpt, 2 * 8 * WW, [[1, 8 * WW]]),
                             sv(tmpt, 3 * 8 * WW, [[1, 8 * WW]]))
        nc.vector.tensor_add(sv(At, 0, [[1, 8 * WW]]),
                             sv(tmpt, 0, [[1, 8 * WW]]),
                             sv(tmpt, 2 * 8 * WW, [[1, 8 * WW]]))
        pso = psO.tile([128, 256], F32, tag="psout")
        for hh in range(2):
            pst = psD.tile([128, 1024], F16, tag="pst")
            if lv == 2:
                for h4 in range(4):
                    h = hh * 4 + h4
                    for cch in range(2):
                        nc.tensor.transpose(
                            pst[:, h4 * 256 + cch * 128: h4 * 256 + (cch + 1) * 128],
                            At[:, h * WW + cch * 128: h * WW + (cch + 1) * 128],
                            identH[:])
                Ah = dn2.tile([128, 1024], F16, tag="Ah")
                nc.scalar.copy(sv(Ah, 0, [[1, 1024]]), sv(pst, 0, [[1, 1024]]))
                for h4 in range(4):
                    h = hh * 4 + h4
                    for cch in range(2):
                        nc.tensor.matmul(
                            pso[:, h * 32:(h + 1) * 32],
                            lhsT=Ah[:, h4 * 256 + cch * 128: h4 * 256 + (cch + 1) * 128],
                            rhs=febf[2][:, cch, :],
                            start=(cch == 0), stop=(cch == 1))
            else:
                for h4 in range(4):
                    h = hh * 4 + h4
                    nc.tensor.transpose(pst[0:64, h4 * 128:(h4 + 1) * 128],
                                        At[:, h * WW:(h + 1) * WW],
                                        identH[:])
                Ah = dn2.tile([128, 1024], F16, tag="Ah")
                nc.scalar.copy(Ah[0:64, 0:512], pst[0:64, 0:512])
                for h4 in range(4):
                    h = hh * 4 + h4
                    nc.tensor.matmul(
                        pso[:, h * 32:(h + 1) * 32],
                        lhsT=Ah[0:64, h4 * 128:(h4 + 1) * 128],
                        rhs=febf[3][0:64, :],
                        start=True, stop=True)
        ob = opool.tile([128, 256], F32, tag="ob")
        nc.vector.tensor_add(ob[:], pso[:],
                             sv(ebt, lv * HD, [[0, 8], [1, 32]]))
        nc.sync.dma_start(
            out=_ap(out, (lv * BPL + qb) * 128 * 256, [[256, 128], [1, 256]]),
            in_=ob[:],
        )

    # ---------------- schedule ----------------
    # descriptor levels first: get the Pool engine (SWDGE) going ASAP
    emit_logits(0)
    emit_femb(0)
    emit_prep_desc(0)
    st0 = [emit_gathers(0, g2) for g2 in range(2)]   # kick first gathers
    emit_logits(1)
    emit_femb(1)
    emit_prep_desc(1)
    # dense-level groundwork (fills PE/ACT/DVE while gathers run)
    emit_femb(2)
    emit_femb(3)
    emit_logits(2)
    emit_logits(3)
    hx2, hy2 = emit_hats(2)
    # interleave: combines as supertiles land, dense qb work between
    emit_combine(0, 0, st0[0])
    st0.append(emit_gathers(0, 2))
    emit_dense_qb(2, 0, hx2, hy2)
    emit_combine(0, 1, st0[1])
    st0.append(emit_gathers(0, 3))
    emit_dense_qb(2, 1, hx2, hy2)
    emit_combine(0, 2, st0[2])
    st1 = [emit_gathers(1, 0)]
    emit_dense_qb(2, 2, hx2, hy2)
    emit_combine(0, 3, st0[3])
    st1.append(emit_gathers(1, 1))
    emit_dense_qb(2, 3, hx2, hy2)
    emit_combine(1, 0, st1[0])
    st1.append(emit_gathers(1, 2))
    emit_dense_qb(2, 4, hx2, hy2)
    emit_combine(1, 1, st1[1])
    st1.append(emit_gathers(1, 3))
    emit_dense_qb(2, 5, hx2, hy2)
    emit_combine(1, 2, st1[2])
    emit_dense_qb(2, 6, hx2, hy2)
    emit_combine(1, 3, st1[3])
    emit_dense_qb(2, 7, hx2, hy2)
    hx3, hy3 = emit_hats(3)
    for qb in range(BPL):
        emit_dense_qb(3, qb, hx3, hy3)


def build_program():
    nc = bacc.Bacc("TRN2", target_bir_lowering=False, debug=False,
                   num_swdge_queues=4)
    io = {}
    io["x"] = nc.dram_tensor("x", [L, P, C], F32, kind="ExternalInput").ap()
    io["ref"] = nc.dram_tensor("ref", [L, P, 2], F32, kind="ExternalInput").ap()
    for i, (H, W) in enumerate(LEVEL_HW):
        io[f"feat{i}"] = nc.dram_tensor(f"feat{i}", [C, H, W], F32,
                                        kind="ExternalInput").ap()
    io["w_attn"] = nc.dram_tensor("w_attn", [C, NH * NS], F32,
                                  kind="ExternalInput").ap()
    io["b_attn"] = nc.dram_tensor("b_attn", [NH * NS], F32,
                                  kind="ExternalInput").ap()
    io["w_off"] = nc.dram_tensor("w_off", [C, 2 * NH * NS], F32,
                                 kind="ExternalInput").ap()
    io["b_off"] = nc.dram_tensor("b_off", [2 * NH * NS], F32,
                                 kind="ExternalInput").ap()
    io["embed_w"] = nc.dram_tensor("embed_w", [L, C, HD], F32,
                                   kind="ExternalInput").ap()
    io["embed_b"] = nc.dram_tensor("embed_b", [L, HD], F32,
                                   kind="ExternalInput").ap()
    io["permP"] = nc.dram_tensor("permP", [128, 128], F32,
                                 kind="ExternalInput").ap()
    io["xg"] = nc.dram_tensor("xg", [16], F32, kind="ExternalInput").ap()
    io["out"] = nc.dram_tensor("out", [L, P, NH * HD], F32,
                               kind="ExternalOutput").ap()
    io["femb4"] = {
        lv: nc.dram_tensor(f"femb4_{lv}", [LEVEL_HW[lv][0] * LEVEL_HW[lv][1], 128],
                           F16, kind="Internal").ap()
        for lv in DESC_LV
    }
    with tile.TileContext(nc) as tc:
        with ExitStack() as ctx:
            emit_kernel(ctx, tc, io)
    nc.compile()
    return nc


_prog = None


def kernel(**inputs):
    global _prog
    if _prog is None:
        _prog = build_program()
    nc = _prog
    res = run_bass_kernel_spmd(nc, _in_maps(inputs), list(range(B)))
    out = np.stack([res.results[i]["out"] for i in range(B)], axis=0)
    return out.reshape(B, L, P, NH * HD)


def _perm_matrix():
    p = np.zeros((128, 128), np.float32)
    for n in range(128):
        p[(n % 8) * 16 + n // 8, n] = 1.0
    return p


def _in_maps(inputs):
    keys = ["x", "ref", "feat0", "feat1", "feat2", "feat3",
            "w_attn", "b_attn", "w_off", "b_off", "embed_w", "embed_b"]
    per_batch = {"x", "ref", "feat0", "feat1", "feat2", "feat3"}
    pm = _perm_matrix()
    xg = np.arange(16, dtype=np.float32)
    maps = []
    for b in range(B):
        m = {"permP": pm, "xg": xg}
        for kk in keys:
            v = np.ascontiguousarray(np.asarray(inputs[kk], dtype=np.float32))
            m[kk] = v[b] if kk in per_batch else v
        maps.append(m)
    return maps


def profile(inputs):
    """Run with tracing; returns HW exec time in ns (or None if unavailable)."""
    global _prog
    if _prog is None:
        _prog = build_program()
    res = run_bass_kernel_spmd(_prog, _in_maps(inputs), list(range(B)), trace=True)
    return res.exec_time_ns


if __name__ == "__main__":
    build_program()
    print("build ok")
